# revision 17
# baseline (speedup 1.0000x reference)
"""Trainium2 Bass kernel for nn_Block_34067680592489.

Computes, for B=32768 independent signals x[b] (length 256):
  mu,reg = small-CNN(x[b])      (conv5+avgpool4+softplus twice, linear, softplus)
  grad   = TtT x - x_b + reg * DtD x
  x_t    = x - gamma * grad,  gamma = softplus(gamma_p)
  out    = middle root of z^3 -(m+x_t) z^2 + (m x_t - 2 gm) z + gm m,  gm = gamma*mu

Device algorithm (per element, normalized to mass=1; p = s - 1/3 lives in
PSUM directly as the matmul accumulation, s = (1+xt)/3):
  E   = 2p - 1/3  (= 2s - 1)
  E2  = E^2                                  (ACT Square of PSUM, table-free)
  hm4 = E2 + c13x4   (= 4(sqe + c13) = -4p/3... the cubic's -p coeff x4)
  C8  = E*(1.5*hm4 - 0.5*E^2 - 1.5)  (= 8C = -4q)    (fused DVE op)
  D4  = hm4^3 - C8^2 (= 64 D' > 0)                   (fused DVE op)
  z|irs = AbsRsqrt(hm4 | D4)       (ONE table for the whole mid-phase)
  r   = hm4 * z   (= 2 sqrt(hm))
  w   = C8 * irs  (= C/sqrt(D'))
  at  = Arctan(w)                            (trig table, tail block)
  root= s - r*sin(at/3)                      (DVE odd-poly * r, + s16)
The reg * DtD x term is folded into the PE contraction: xr = reg (.) x is
built once on DVE (bf16, 2x mode) from a PE-broadcast of the CNN's reg
output, so no per-tile scaling passes are needed; x_b enters PSUM via one
eye-stationary identity matmul per tile.

Sharding: pure data parallel over batch, 8 cores x 4096 rows.
"""

import numpy as np

B_TOTAL = 32768
N = 256
N_CORES = 8
BC = B_TOTAL // N_CORES      # rows per core
TILES = BC // 128            # 32 batch tiles of 128
CT = 4                       # tiles per elementwise chunk
CHUNKS = TILES // CT         # 8
CF = CT * N                  # chunk free size (1024)

_PROG = {}


def _np_f32(a):
    return np.ascontiguousarray(np.asarray(a, dtype=np.float32))


def _conv_pool_mat(w, L):
    """(L/4, L) matrix implementing conv1d(k=5,pad=2) then avgpool4."""
    taps = np.asarray(w, np.float32).reshape(5)
    C = np.zeros((L, L), np.float32)
    for n in range(L):
        for k in range(5):
            m = n + k - 2
            if 0 <= m < L:
                C[n, m] = taps[k]
    P = np.zeros((L // 4, L), np.float32)
    for i in range(L // 4):
        P[i, 4 * i:4 * i + 4] = 0.25
    return (P @ C).astype(np.float32)


_CUSTOM_OPS = {}


def _get_custom_ops():
    """Register this kernel's fused custom-DVE ops (idempotent).

    CH8:  C8 = (in0*s1 - E^2*imm2 - s1)*E,  E = 2*in1 - s0
          (called with in0=hm4, in1=p(PSUM), s0=1/3, s1=1.5, imm2=0.5)
    DQ:   D4 = in0^3 - in1^2                 (in0=hm4, in1=C8)
    RGSIN: rg = (at*(s0+at^2*(s1+at^2*imm2)))*r   (odd -sin(x/3) poly)
    """
    if _CUSTOM_OPS:
        return _CUSTOM_OPS
    import concourse.dve_ops as dops
    from concourse.dve_spec import (Spec, Src0, Src1, C0, C1, C2, sq,
                                    lower, _has_src1)
    from concourse.dve_uop import DveOpSpec

    def reg(name, spec):
        if name in dops._SUB_OPCODE_FOR_NAME:
            return next(o for o in dops.OPS if o.name == name)
        row = dops._CUSTOM_DVE_ROW_BASE + len(dops.OPS)
        assert row < 0x20
        dops._SUB_OPCODE_FOR_NAME[name] = row
        shas = {}
        for ver in ("v3", "v4"):
            u = lower(spec, ver=ver)
            shas[ver] = DveOpSpec(name=name, opcode=row, uops=u,
                                  rd1_en=_has_src1(spec)).sha(ver)
        op = dops.DveOp(name, spec, subdim=False, uops_sha=shas)
        dops.OPS.append(op)
        dops.CUSTOM_DVE_SPECS[name] = spec
        return op

    import numpy as np_

    _E = (Src1 + Src1) - C0
    _CUSTOM_OPS['CH8'] = reg('ANT_K_CH8', Spec(
        body=((Src0 * C1) - sq(_E) * C2 - C1) * _E,
        reference=lambda in0, in1, s0, s1, imm2:
            (((in0 * s1) - (2.0 * in1 - s0) ** 2 * imm2 - s1)
             * (2.0 * in1 - s0)).astype(np_.float32),
    ))
    _CUSTOM_OPS['DQ'] = reg('ANT_K_DQ', Spec(
        body=(sq(Src0) * Src0) - sq(Src1),
        reference=lambda in0, in1, s0, s1, imm2:
            (in0 ** 2 * in0 - in1 ** 2).astype(np_.float32),
    ))
    _a2 = sq(Src0)
    _CUSTOM_OPS['RGSIN'] = reg('ANT_K_RGSIN', Spec(
        body=(Src0 * (C0 + _a2 * (C1 + _a2 * C2))) * Src1,
        reference=lambda in0, in1, s0, s1, imm2:
            ((in0 * (s0 + in0 * in0 * (s1 + in0 * in0 * imm2))) * in1
             ).astype(np_.float32),
    ))
    return _CUSTOM_OPS


_TABLES_PATCHED = False


def _patch_act_tables():
    """Restrict ACT table-set choice to the sets this kernel uses."""
    global _TABLES_PATCHED
    if _TABLES_PATCHED:
        return
    import concourse.bacc as bacc
    keep = {'natural_log_exp_and_others',
            'abs_reciprocal_sqrt_and_small', 'trig_and_small'}
    orig = bacc.get_activation_tables

    def patched(arch):
        t = orig(arch)
        return {k: (v if k in keep else set()) for k, v in t.items()}

    bacc.get_activation_tables = patched
    _TABLES_PATCHED = True


def _build_program():
    import concourse.bacc as bacc
    import concourse.tile as tile
    import concourse.mybir as mybir
    from concourse.tile import add_dep_helper
    _patch_act_tables()

    dt = mybir.dt
    f32 = dt.float32
    bf16 = dt.bfloat16
    fp16 = dt.float16
    Alu = mybir.AluOpType
    AF = mybir.ActivationFunctionType
    odt = bf16

    COPS = _get_custom_ops()
    nc = bacc.Bacc("TRN2", target_bir_lowering=False, debug=False,
                   num_devices=N_CORES)

    XT = nc.dram_tensor("xt", (256, BC), bf16, kind="ExternalInput")
    XB = nc.dram_tensor("xb", (BC, 256), bf16, kind="ExternalInput")
    EYE = nc.dram_tensor("eye", (128, 128), bf16, kind="ExternalInput")
    WM = nc.dram_tensor("wm", (256, 512), bf16, kind="ExternalInput")
    M1T = nc.dram_tensor("m1t", (256, 128), bf16, kind="ExternalInput")
    M2BD = nc.dram_tensor("m2bd", (128, 32), bf16, kind="ExternalInput")
    LWBD = nc.dram_tensor("lwbd", (128, 2), bf16, kind="ExternalInput")
    B2V = nc.dram_tensor("b2v", (128, 1), f32, kind="ExternalInput")
    B3V = nc.dram_tensor("b3v", (128, 1), f32, kind="ExternalInput")
    LBM = nc.dram_tensor("lbm", (128, 1), f32, kind="ExternalInput")
    LBR = nc.dram_tensor("lbr", (128, 1), f32, kind="ExternalInput")
    GSC = nc.dram_tensor("gsc", (128, 1), f32, kind="ExternalInput")
    ONESR = nc.dram_tensor("onesr", (1, 128), bf16, kind="ExternalInput")
    OUT = nc.dram_tensor("out", (BC, 256), odt, kind="ExternalOutput")

    NSG = 2                          # supergroups
    GPS = CHUNKS // NSG              # groups per supergroup

    with tile.TileContext(nc) as tc:
        with (
            tc.tile_pool(name="const", bufs=1) as cpool,
            tc.tile_pool(name="xin", bufs=1) as xtpool,
            tc.tile_pool(name="hd", bufs=3) as hdpool,
            tc.tile_pool(name="c8", bufs=3) as c8pool,
            tc.tile_pool(name="e2", bufs=2) as e2pool,
            tc.tile_pool(name="s16", bufs=CHUNKS) as s16pool,
            tc.tile_pool(name="rr", bufs=CHUNKS) as rrpool,
            tc.tile_pool(name="wv", bufs=CHUNKS) as wvpool,
            tc.tile_pool(name="rg", bufs=3) as rgpool,
            tc.tile_pool(name="oo", bufs=3) as oopool,
        ):
            # ---- constants into SBUF ----
            wm = cpool.tile([128, 2, 512], bf16)
            m1t = cpool.tile([128, 2, 128], bf16)
            m2bd = cpool.tile([128, 32], bf16)
            lwbd = cpool.tile([128, 2], bf16)
            b2v = cpool.tile([128, 1], f32)
            b3v = cpool.tile([128, 1], f32)
            lbm = cpool.tile([128, 1], f32)
            lbr = cpool.tile([128, 1], f32)
            gsc = cpool.tile([128, 1], f32)
            spE = cpool.tile([128, 2 * TILES], f32)
            sp = cpool.tile([128, 2 * TILES], f32)
            gph = cpool.tile([128, TILES], f32)
            c13x4 = cpool.tile([128, TILES], f32)
            eye = cpool.tile([128, 128], bf16)
            onesr = cpool.tile([1, 128], bf16)
            regc = cpool.tile([128, TILES], bf16)
            regT = [cpool.tile([16, 128], bf16, name=f"regT{i}")
                    for i in range(NSG)]
            regf = cpool.tile([1, BC], bf16)
            regB = cpool.tile([128, BC], bf16)
            epsv = cpool.tile([128, 1], f32)
            nc.vector.memset(epsv[:], 1e-5)
            cm13 = cpool.tile([128, 1], f32)
            nc.vector.memset(cm13[:], -1.0 / 3.0)
            c13v = cpool.tile([128, 1], f32)
            nc.vector.memset(c13v[:], 1.0 / 3.0)
            nc.sync.dma_start(eye[:], EYE[:])
            nc.sync.dma_start(onesr[:], ONESR[:])
            for k in range(2):
                nc.sync.dma_start(m1t[:, k, :], M1T[128 * k:128 * (k + 1), :])
            nc.sync.dma_start(m2bd[:], M2BD[:])
            nc.sync.dma_start(lwbd[:], LWBD[:])
            nc.sync.dma_start(b2v[:], B2V[:])
            nc.sync.dma_start(b3v[:], B3V[:])
            nc.sync.dma_start(lbm[:], LBM[:])
            nc.sync.dma_start(lbr[:], LBR[:])
            nc.sync.dma_start(gsc[:], GSC[:])

            # ---- inputs ----
            xt_sb = xtpool.tile([128, 2, BC], bf16)
            xr_sb = xtpool.tile([128, 2, BC], bf16)
            xb_sb = xtpool.tile([128, TILES, 256], bf16)
            XBv = XB[:].rearrange("(t p) n -> p t n", p=128)
            for qq in range(8):
                qsl = slice(BC // 8 * qq, BC // 8 * (qq + 1))
                tsl = slice(TILES // 8 * qq, TILES // 8 * (qq + 1))
                for k in range(2):
                    nc.sync.dma_start(xt_sb[:, k, qsl],
                                      XT[128 * k:128 * (k + 1), qsl])
                nc.gpsimd.dma_start(xb_sb[:, tsl, :], XBv[:, tsl, :])
                if qq == 0:
                    for k in range(2):
                        nc.scalar.dma_start(wm[:, k, :],
                                            WM[128 * k:128 * (k + 1), :])

            sp_insts = []
            spEv = spE[:].rearrange("p (t c) -> p c t", c=2)
            spv = sp[:].rearrange("p (t c) -> p c t", c=2)

            # ================= CNN + reg-broadcast phase =================
            with (
                tc.tile_pool(name="cnn", bufs=2) as cnnpool,
                tc.tile_pool(name="pc1", bufs=2, space="PSUM") as pc1pool,
                tc.tile_pool(name="pc2", bufs=1, space="PSUM") as pc2pool,
                tc.tile_pool(name="pc3", bufs=1, space="PSUM") as pc3pool,
                tc.tile_pool(name="pcb", bufs=2, space="PSUM") as pcbpool,
            ):
                p3 = pc3pool.tile([128, 2 * TILES], f32)
                for sg in range(NSG):
                    gs_range = range(GPS * sg, GPS * (sg + 1))
                    p2 = pc2pool.tile([128, 512], f32, tag="p2",
                                      name=f"p2sg{sg}")
                    for q, g in enumerate(gs_range):
                        sl = slice(512 * g, 512 * (g + 1))
                        p1 = pc1pool.tile([128, 512], f32, tag="p1",
                                          name=f"p1g{g}")
                        nc.tensor.matmul(p1[:], m1t[:, 0, :],
                                         xt_sb[:, 0, sl],
                                         start=True, stop=False)
                        nc.tensor.matmul(p1[:], m1t[:, 1, :],
                                         xt_sb[:, 1, sl],
                                         start=False, stop=True)
                        eh1 = cnnpool.tile([128, 512], f32, tag="eh1",
                                           name=f"eh1g{g}")
                        nc.scalar.activation(eh1[:], p1[:], AF.Exp,
                                             bias=b2v[:])
                        h1s = cnnpool.tile([128, 512], bf16, tag="h1s",
                                           name=f"h1sg{g}")
                        nc.scalar.activation(h1s[:], eh1[:], AF.Ln, bias=1.0)
                        nc.tensor.matmul(p2[32 * q:32 * (q + 1), :],
                                         m2bd[:], h1s[:],
                                         start=True, stop=True,
                                         tile_position=(0, 32 * q),
                                         skip_group_check=True)
                    eh2 = cnnpool.tile([128, 512], f32, tag="eh2",
                                       name=f"eh2sg{sg}")
                    nc.scalar.activation(eh2[:], p2[:], AF.Exp,
                                         bias=b3v[:])
                    h2s = cnnpool.tile([128, 512], bf16, tag="h2s",
                                       name=f"h2ssg{sg}")
                    nc.scalar.activation(h2s[:], eh2[:], AF.Ln, bias=1.0)
                    for q, g in enumerate(gs_range):
                        for i in range(4):
                            t = 4 * g + i
                            nc.tensor.matmul(
                                p3[:, 2 * t:2 * t + 2],
                                h2s[32 * q:32 * (q + 1),
                                    128 * i:128 * (i + 1)],
                                lwbd[32 * q:32 * (q + 1), :],
                                start=True, stop=True,
                                tile_position=(32 * q, 0),
                                skip_group_check=True)

                    sgt = slice(4 * GPS * sg, 4 * GPS * (sg + 1))
                    sgs = slice(8 * GPS * sg, 8 * GPS * (sg + 1))
                    nc.scalar.activation(spEv[:, 0, sgt],
                                         p3[:, sgs].rearrange(
                                             "p (t c) -> p c t", c=2)[:, 0, :],
                                         AF.Exp, bias=lbm[:])
                    nc.scalar.activation(spEv[:, 1, sgt],
                                         p3[:, sgs].rearrange(
                                             "p (t c) -> p c t", c=2)[:, 1, :],
                                         AF.Exp, bias=lbr[:])
                    sp_i = nc.scalar.activation(sp[:, sgs], spE[:, sgs],
                                                AF.Ln, bias=1.0)
                    sp_insts.append(sp_i)
                    nc.vector.tensor_scalar(gph[:, sgt], spv[:, 0, sgt],
                                            gsc[:], -0.25,
                                            Alu.mult, Alu.add)
                    nc.vector.tensor_scalar(c13x4[:, sgt], gph[:, sgt],
                                            8.0 / 3.0, 1.0,
                                            Alu.mult, Alu.add)
                    # bf16 copy of reg for the PE transpose
                    nc.vector.tensor_scalar(regc[:, sgt], spv[:, 1, sgt],
                                            1.0, None, Alu.mult)

                    # reg (this supergroup's half) -> regB broadcast along
                    # partitions, via PE transpose + 1-contract broadcast
                    # matmuls; then xr = reg (.) x for this batch half.
                    hsl = slice(BC // NSG * sg, BC // NSG * (sg + 1))
                    pT = pcbpool.tile([16, 128], f32, tag="pt",
                                      name=f"pT{sg}")
                    nc.tensor.matmul(pT[:], regc[:, sgt], eye[:],
                                     start=True, stop=True)
                    nc.vector.tensor_scalar(regT[sg][:], pT[:], 1.0, None,
                                            Alu.mult)
                    nc.sync.dma_start(
                        regf[:, hsl].rearrange("o (t n) -> o t n", n=128),
                        regT[sg][:].rearrange("t n -> t () n"))
                    for tt_ in range(TILES // NSG):
                        t = TILES // NSG * sg + tt_
                        bb = t // 4
                        pB = (pcbpool.tile([128, 512], f32, tag="pb",
                                           name=f"pB{bb}")
                              if t % 4 == 0 else pB)
                        nc.tensor.matmul(
                            pB[:, 128 * (t % 4):128 * (t % 4 + 1)],
                            onesr[:], regf[0:1, 128 * t:128 * (t + 1)],
                            start=(t % 4 == 0), stop=(t % 4 == 3),
                            skip_group_check=True)
                        if t % 4 == 3:
                            nc.vector.tensor_scalar(
                                regB[:, 512 * bb:512 * (bb + 1)], pB[:],
                                1.0, None, Alu.mult)
                    for k in range(2):
                        nc.vector.tensor_tensor(xr_sb[:, k, hsl],
                                                xt_sb[:, k, hsl],
                                                regB[:, hsl], Alu.mult)

            # ================= main phase =================
            s16_chunks = [None] * CHUNKS
            r_chunks = [None] * CHUNKS
            w_chunks = [None] * CHUNKS
            last_zd = None
            with tc.tile_pool(name="pm", bufs=3, space="PSUM") as pmpool:
                for c in range(CHUNKS):
                    pm = pmpool.tile([128, CF], f32, tag="pm", name=f"pm{c}")
                    # identity moves first (eye stationary; start=True resets
                    # each 512-col bank, so issue per-bank leader first)
                    for i in range(CT):
                        t = CT * c + i
                        nc.tensor.matmul(pm[:, 256 * i:256 * (i + 1)],
                                         eye[:], xb_sb[:, t, :],
                                         start=(i % 2 == 0), stop=False,
                                         skip_group_check=True)
                    for i in range(CT):
                        t = CT * c + i
                        tsl = slice(128 * t, 128 * (t + 1))
                        osl = slice(256 * i, 256 * (i + 1))
                        for k in range(2):
                            nc.tensor.matmul(
                                pm[:, osl], xt_sb[:, k, tsl],
                                wm[:, k, 0:256],
                                start=False, stop=False,
                                skip_group_check=True)
                            nc.tensor.matmul(
                                pm[:, osl], xr_sb[:, k, tsl],
                                wm[:, k, 256:512],
                                start=False, stop=(k == 1),
                                skip_group_check=True)
                    # E2 = (2p - 1/3)^2  [ACT, table-free]
                    e2 = e2pool.tile([128, CF], fp16, tag="e2",
                                     name=f"e2c{c}")
                    nc.scalar.activation(e2[:], pm[:], AF.Square,
                                         scale=2.0, bias=cm13[:])
                    # hm4 = E2 + c13x4 (per-tile scalar; fp16 4x mode)
                    hd = hdpool.tile([128, 2, CF], fp16, tag="hd",
                                     name=f"hd{c}")
                    for i in range(CT):
                        t = CT * c + i
                        osl = slice(256 * i, 256 * (i + 1))
                        nc.vector.tensor_scalar(hd[:, 0, osl], e2[:, osl],
                                                c13x4[:, t:t + 1], None,
                                                Alu.add)
                    # C8, D4
                    c8 = c8pool.tile([128, CF], fp16, tag="c8",
                                     name=f"c8c{c}")
                    nc.vector._custom_dve(
                        COPS['CH8'], out=c8[:], in0=hd[:, 0, :], in1=pm[:],
                        s0=1.0 / 3.0, s1=1.5, imm2=0.5)
                    nc.vector._custom_dve(
                        COPS['DQ'], out=hd[:, 1, :], in0=hd[:, 0, :],
                        in1=c8[:])
                    # z | irs = AbsRsqrt(hm4 | D4)  [one 2048-wide ACT op]
                    zd = rgpool.tile([128, 2, CF], fp16, tag="zd",
                                     name=f"zd{c}")
                    zd_i = nc.scalar.activation(
                        zd[:].rearrange("p a b -> p (a b)"),
                        hd[:].rearrange("p a b -> p (a b)"),
                        AF.Abs_reciprocal_sqrt, bias=epsv[:])
                    if last_zd is None:
                        for spi in sp_insts:
                            add_dep_helper(zd_i.ins, spi.ins, sync=True,
                                           reason="absrsqrt after NLE table")
                    else:
                        add_dep_helper(zd_i.ins, last_zd.ins, sync=False,
                                       reason="chain zd block")
                    last_zd = zd_i
                    # r = hm4*z,  w = C8*irs  (all-SBUF: Pool can take them)
                    r = rrpool.tile([128, CF], fp16, tag="rr", name=f"r{c}")
                    r_chunks[c] = r
                    nc.gpsimd.tensor_tensor(r[:], hd[:, 0, :], zd[:, 0, :],
                                            Alu.mult)
                    w = wvpool.tile([128, CF], fp16, tag="wv", name=f"w{c}")
                    w_chunks[c] = w
                    nc.gpsimd.tensor_tensor(w[:], c8[:], zd[:, 1, :],
                                            Alu.mult)
                    # s16 = p + 1/3 (frees the PSUM bank pair; GPSIMD can't
                    # read PSUM, so split between DVE and ACT-Identity)
                    s16 = s16pool.tile([128, CF], fp16, tag="s16",
                                       name=f"s16c{c}")
                    s16_chunks[c] = s16
                    if c % 2 == 0:
                        nc.vector.tensor_scalar(s16[:], pm[:], 1.0 / 3.0,
                                                None, Alu.add)
                    else:
                        nc.scalar.activation(s16[:], pm[:], AF.Identity,
                                             bias=c13v[:])

            # ---- trig tail: arctan block, RGSIN, add, DMA out ----
            last_at = None
            for c in range(CHUNKS):
                w = w_chunks[c]
                at_i = nc.scalar.activation(w[:], w[:], AF.Arctan)
                if last_at is None:
                    add_dep_helper(at_i.ins, last_zd.ins, sync=True,
                                   reason="trig block after absrsqrt block")
                else:
                    add_dep_helper(at_i.ins, last_at.ins, sync=False,
                                   reason="chain trig block")
                last_at = at_i
                rg = rgpool.tile([128, CF], fp16, tag="rg", name=f"rgc{c}")
                nc.vector._custom_dve(
                    COPS['RGSIN'], out=rg[:], in0=w[:],
                    in1=r_chunks[c][:],
                    s0=-1.0 / 3.0, s1=1.0 / 162.0, imm2=-1.0 / 29160.0)
                ot = oopool.tile([128, CF], odt, tag="oo", name=f"o{c}")
                nc.vector.tensor_tensor(ot[:], rg[:], s16_chunks[c][:],
                                        Alu.add)
                dview = OUT[512 * c:512 * (c + 1), :].rearrange(
                    "(tt p) n -> p tt n", p=128)
                nc.sync.dma_start(
                    dview, ot[:].rearrange("p (tt n) -> p tt n", n=256))

    nc.compile()
    return nc


def _get_program():
    key = (B_TOTAL, N, N_CORES)
    if key not in _PROG:
        _PROG[key] = _build_program()
    return _PROG[key]


def _host_prep(inputs):
    import ml_dtypes
    bf = ml_dtypes.bfloat16
    x = _np_f32(inputs['x']).reshape(B_TOTAL, N)
    x_b = _np_f32(inputs['x_b']).reshape(B_TOTAL, N)
    m = float(np.asarray(inputs['mass']).reshape(-1)[0])
    gp = float(np.asarray(inputs['gamma_p']).reshape(-1)[0])
    gamma = float(np.log1p(np.exp(gp))) if gp < 30 else gp
    TtT = _np_f32(inputs['TtT'])
    DtD = _np_f32(inputs['DtD'])

    W_A = ((np.eye(N, dtype=np.float32) - np.float32(gamma) * TtT.T)
           / np.float32(3.0 * m)).astype(np.float32)
    W_B = (-np.float32(gamma) * DtD.T / np.float32(3.0 * m)).astype(np.float32)
    WM = np.concatenate([W_A, W_B], axis=1).astype(bf)          # (256,512)

    M1s, M2s, lws = {}, {}, {}
    for tag in ('mu', 'reg'):
        M1s[tag] = _conv_pool_mat(inputs['w2_' + tag], 256)      # (64,256)
        M2s[tag] = _conv_pool_mat(inputs['w3_' + tag], 64)       # (16,64)
        lws[tag] = _np_f32(inputs['lw_' + tag]).reshape(16)
    M1cat = np.concatenate([M1s['mu'], M1s['reg']], axis=0)      # (128,256)
    M1T = np.ascontiguousarray(M1cat.T).astype(bf)               # (256,128)
    M2BD = np.zeros((128, 32), np.float32)
    M2BD[0:64, 0:16] = M2s['mu'].T
    M2BD[64:128, 16:32] = M2s['reg'].T
    M2BD = M2BD.astype(bf)
    LWBD1 = np.zeros((32, 2), np.float32)
    LWBD1[0:16, 0] = lws['mu']
    LWBD1[16:32, 1] = lws['reg']
    LWBD = np.tile(LWBD1, (4, 1)).astype(bf)                     # (128,2)

    def sc(name):
        return float(np.asarray(inputs[name]).reshape(-1)[0])

    B2V = np.full((128, 1), sc('b2_mu'), np.float32)
    B2V[64:] = sc('b2_reg')
    B3V1 = np.full((32, 1), sc('b3_mu'), np.float32)
    B3V1[16:] = sc('b3_reg')
    B3V = np.tile(B3V1, (4, 1))                                  # (128,1)
    LBM = np.full((128, 1), sc('lb_mu'), np.float32)
    LBR = np.full((128, 1), sc('lb_reg'), np.float32)
    GSC = np.full((128, 1), gamma / (m * m), np.float32)

    EYEM = np.eye(128, dtype=np.float32).astype(bf)
    ONES = np.ones((1, 128), np.float32).astype(bf)
    consts = dict(wm=WM, m1t=M1T, m2bd=M2BD, lwbd=LWBD, eye=EYEM,
                  onesr=ONES, b2v=B2V, b3v=B3V, lbm=LBM, lbr=LBR, gsc=GSC)

    xb3 = (np.float32(gamma / (3.0 * m)) * x_b).astype(bf)
    xbf = x.astype(bf)
    in_maps = []
    for c in range(N_CORES):
        rows = slice(BC * c, BC * (c + 1))
        im = dict(consts)
        im['xt'] = np.ascontiguousarray(xbf[rows].T)
        im['xb'] = np.ascontiguousarray(xb3[rows])
        in_maps.append(im)
    return in_maps, m


def kernel(**inputs) -> np.ndarray:
    from concourse import bass_utils
    nc = _get_program()
    in_maps, m = _host_prep(inputs)
    res = bass_utils.run_bass_kernel_spmd(nc, in_maps,
                                          core_ids=list(range(N_CORES)))
    out = np.concatenate([res.results[c]['out'] for c in range(N_CORES)],
                         axis=0).astype(np.float32)
    if m != 1.0:
        out = (np.float32(m) * out).astype(np.float32)
    return np.ascontiguousarray(out.reshape(B_TOTAL, 1, N))


# revision 18
# speedup vs baseline: 1.0784x; 1.0784x over previous
"""Trainium2 Bass kernel for nn_Block_34067680592489.

Computes, for B=32768 independent signals x[b] (length 256):
  mu,reg = small-CNN(x[b])      (conv5+avgpool4+softplus twice, linear, softplus)
  grad   = TtT x - x_b + reg * DtD x
  x_t    = x - gamma * grad,  gamma = softplus(gamma_p)
  out    = middle root of z^3 -(m+x_t) z^2 + (m x_t - 2 gm) z + gm m,  gm = gamma*mu

Device algorithm (per element, normalized to mass=1; p = s - 1/3 lives in
PSUM directly as the matmul accumulation, s = (1+xt)/3):
  E   = 2p - 1/3  (= 2s - 1)
  E2  = E^2                                  (ACT Square of PSUM, table-free)
  hm4 = E2 + c13x4   (= 4(sqe + c13) = -4p/3... the cubic's -p coeff x4)
  C8  = E*(1.5*hm4 - 0.5*E^2 - 1.5)  (= 8C = -4q)    (fused DVE op)
  D4  = hm4^3 - C8^2 (= 64 D' > 0)                   (fused DVE op)
  z|irs = AbsRsqrt(hm4 | D4)       (ONE table for the whole mid-phase)
  r   = hm4 * z   (= 2 sqrt(hm))
  w   = C8 * irs  (= C/sqrt(D'))
  at  = Arctan(w)                            (trig table, tail block)
  root= s - r*sin(at/3)                      (DVE odd-poly * r, + s16)
The reg * DtD x term is folded into the PE contraction: xr = reg (.) x is
built once on DVE (bf16, 2x mode) from a PE-broadcast of the CNN's reg
output, so no per-tile scaling passes are needed; x_b enters PSUM via one
eye-stationary identity matmul per tile.

Sharding: pure data parallel over batch, 8 cores x 4096 rows.
"""

import numpy as np

B_TOTAL = 32768
N = 256
N_CORES = 8
BC = B_TOTAL // N_CORES      # rows per core
TILES = BC // 128            # 32 batch tiles of 128
CT = 4                       # tiles per elementwise chunk
CHUNKS = TILES // CT         # 8
CF = CT * N                  # chunk free size (1024)

_PROG = {}


def _np_f32(a):
    return np.ascontiguousarray(np.asarray(a, dtype=np.float32))


def _conv_pool_mat(w, L):
    """(L/4, L) matrix implementing conv1d(k=5,pad=2) then avgpool4."""
    taps = np.asarray(w, np.float32).reshape(5)
    C = np.zeros((L, L), np.float32)
    for n in range(L):
        for k in range(5):
            m = n + k - 2
            if 0 <= m < L:
                C[n, m] = taps[k]
    P = np.zeros((L // 4, L), np.float32)
    for i in range(L // 4):
        P[i, 4 * i:4 * i + 4] = 0.25
    return (P @ C).astype(np.float32)


_CUSTOM_OPS = {}


def _get_custom_ops():
    """Register this kernel's fused custom-DVE ops (idempotent).

    CH8:  C8 = (in0*s1 - E^2*imm2 - s1)*E,  E = 2*in1 - s0
          (called with in0=hm4, in1=p(PSUM), s0=1/3, s1=1.5, imm2=0.5)
    DQ:   D4 = in0^3 - in1^2                 (in0=hm4, in1=C8)
    RGSIN: rg = (at*(s0+at^2*(s1+at^2*imm2)))*r   (odd -sin(x/3) poly)
    """
    if _CUSTOM_OPS:
        return _CUSTOM_OPS
    import concourse.dve_ops as dops
    from concourse.dve_spec import (Spec, Src0, Src1, C0, C1, C2, sq,
                                    lower, _has_src1)
    from concourse.dve_uop import DveOpSpec

    def reg(name, spec):
        if name in dops._SUB_OPCODE_FOR_NAME:
            return next(o for o in dops.OPS if o.name == name)
        row = dops._CUSTOM_DVE_ROW_BASE + len(dops.OPS)
        assert row < 0x20
        dops._SUB_OPCODE_FOR_NAME[name] = row
        shas = {}
        for ver in ("v3", "v4"):
            u = lower(spec, ver=ver)
            shas[ver] = DveOpSpec(name=name, opcode=row, uops=u,
                                  rd1_en=_has_src1(spec)).sha(ver)
        op = dops.DveOp(name, spec, subdim=False, uops_sha=shas)
        dops.OPS.append(op)
        dops.CUSTOM_DVE_SPECS[name] = spec
        return op

    import numpy as np_

    _E = (Src1 + Src1) - C0
    _CUSTOM_OPS['CH8'] = reg('ANT_K_CH8', Spec(
        body=((Src0 * C1) - sq(_E) * C2 - C1) * _E,
        reference=lambda in0, in1, s0, s1, imm2:
            (((in0 * s1) - (2.0 * in1 - s0) ** 2 * imm2 - s1)
             * (2.0 * in1 - s0)).astype(np_.float32),
    ))
    _CUSTOM_OPS['DQ'] = reg('ANT_K_DQ', Spec(
        body=(sq(Src0) * Src0) - sq(Src1),
        reference=lambda in0, in1, s0, s1, imm2:
            (in0 ** 2 * in0 - in1 ** 2).astype(np_.float32),
    ))
    _a2 = sq(Src0)
    _CUSTOM_OPS['RGSIN'] = reg('ANT_K_RGSIN', Spec(
        body=(Src0 * (C0 + _a2 * (C1 + _a2 * C2))) * Src1,
        reference=lambda in0, in1, s0, s1, imm2:
            ((in0 * (s0 + in0 * in0 * (s1 + in0 * in0 * imm2))) * in1
             ).astype(np_.float32),
    ))
    return _CUSTOM_OPS


_TABLES_PATCHED = False


def _patch_act_tables():
    """Restrict ACT table-set choice to the sets this kernel uses."""
    global _TABLES_PATCHED
    if _TABLES_PATCHED:
        return
    import concourse.bacc as bacc
    keep = {'natural_log_exp_and_others',
            'abs_reciprocal_sqrt_and_small', 'trig_and_small'}
    orig = bacc.get_activation_tables

    def patched(arch):
        t = orig(arch)
        return {k: (v if k in keep else set()) for k, v in t.items()}

    bacc.get_activation_tables = patched
    _TABLES_PATCHED = True


def _build_program():
    import concourse.bacc as bacc
    import concourse.tile as tile
    import concourse.mybir as mybir
    from concourse.tile import add_dep_helper
    _patch_act_tables()

    dt = mybir.dt
    f32 = dt.float32
    bf16 = dt.bfloat16
    fp16 = dt.float16
    Alu = mybir.AluOpType
    AF = mybir.ActivationFunctionType
    odt = bf16

    COPS = _get_custom_ops()
    nc = bacc.Bacc("TRN2", target_bir_lowering=False, debug=False,
                   num_devices=N_CORES)

    XT = nc.dram_tensor("xt", (256, BC), bf16, kind="ExternalInput")
    XB = nc.dram_tensor("xb", (BC, 256), bf16, kind="ExternalInput")
    EYE = nc.dram_tensor("eye", (128, 128), bf16, kind="ExternalInput")
    WM = nc.dram_tensor("wm", (256, 512), bf16, kind="ExternalInput")
    M1T = nc.dram_tensor("m1t", (256, 128), bf16, kind="ExternalInput")
    M2BD = nc.dram_tensor("m2bd", (128, 32), bf16, kind="ExternalInput")
    LWBD = nc.dram_tensor("lwbd", (128, 2), bf16, kind="ExternalInput")
    B2V = nc.dram_tensor("b2v", (128, 1), f32, kind="ExternalInput")
    B3V = nc.dram_tensor("b3v", (128, 1), f32, kind="ExternalInput")
    LBM = nc.dram_tensor("lbm", (128, 1), f32, kind="ExternalInput")
    LBR = nc.dram_tensor("lbr", (128, 1), f32, kind="ExternalInput")
    GSC = nc.dram_tensor("gsc", (128, 1), f32, kind="ExternalInput")
    ONESR = nc.dram_tensor("onesr", (1, 128), bf16, kind="ExternalInput")
    OUT = nc.dram_tensor("out", (BC, 256), odt, kind="ExternalOutput")

    NSG = 2                          # supergroups
    GPS = CHUNKS // NSG              # groups per supergroup

    with tile.TileContext(nc) as tc:
        with (
            tc.tile_pool(name="const", bufs=1) as cpool,
            tc.tile_pool(name="xin", bufs=1) as xtpool,
            tc.tile_pool(name="hd", bufs=3) as hdpool,
            tc.tile_pool(name="c8", bufs=3) as c8pool,
            tc.tile_pool(name="e2", bufs=2) as e2pool,
            tc.tile_pool(name="s16", bufs=CHUNKS) as s16pool,
            tc.tile_pool(name="rr", bufs=CHUNKS) as rrpool,
            tc.tile_pool(name="wv", bufs=CHUNKS) as wvpool,
            tc.tile_pool(name="rg", bufs=3) as rgpool,
            tc.tile_pool(name="oo", bufs=3) as oopool,
        ):
            # ---- constants into SBUF ----
            wm = cpool.tile([128, 2, 512], bf16)
            m1t = cpool.tile([128, 2, 128], bf16)
            m2bd = cpool.tile([128, 32], bf16)
            lwbd = cpool.tile([128, 2], bf16)
            b2v = cpool.tile([128, 1], f32)
            b3v = cpool.tile([128, 1], f32)
            lbm = cpool.tile([128, 1], f32)
            lbr = cpool.tile([128, 1], f32)
            gsc = cpool.tile([128, 1], f32)
            spE = cpool.tile([128, 2 * TILES], f32)
            sp = cpool.tile([128, 2 * TILES], f32)
            gph = cpool.tile([128, TILES], f32)
            c13x4 = cpool.tile([128, TILES], f32)
            eye = cpool.tile([128, 128], bf16)
            onesr = cpool.tile([1, 128], bf16)
            regc = cpool.tile([128, TILES], bf16)
            regT = [cpool.tile([16, 128], bf16, name=f"regT{i}")
                    for i in range(NSG)]
            regf = cpool.tile([1, BC], bf16)
            regB = cpool.tile([128, BC], bf16)
            epsv = cpool.tile([128, 1], f32)
            nc.vector.memset(epsv[:], 1e-5)
            cm13 = cpool.tile([128, 1], f32)
            nc.vector.memset(cm13[:], -1.0 / 3.0)
            c13v = cpool.tile([128, 1], f32)
            nc.vector.memset(c13v[:], 1.0 / 3.0)
            nc.sync.dma_start(eye[:], EYE[:])
            nc.sync.dma_start(onesr[:], ONESR[:])
            for k in range(2):
                nc.sync.dma_start(m1t[:, k, :], M1T[128 * k:128 * (k + 1), :])
            nc.sync.dma_start(m2bd[:], M2BD[:])
            nc.sync.dma_start(lwbd[:], LWBD[:])
            nc.sync.dma_start(b2v[:], B2V[:])
            nc.sync.dma_start(b3v[:], B3V[:])
            nc.sync.dma_start(lbm[:], LBM[:])
            nc.sync.dma_start(lbr[:], LBR[:])
            nc.sync.dma_start(gsc[:], GSC[:])

            # ---- inputs ----
            xt_sb = xtpool.tile([128, 2, BC], bf16)
            xr_sb = xtpool.tile([128, 2, BC], bf16)
            xb_sb = xtpool.tile([128, TILES, 256], bf16)
            XBv = XB[:].rearrange("(t p) n -> p t n", p=128)
            for qq in range(8):
                qsl = slice(BC // 8 * qq, BC // 8 * (qq + 1))
                tsl = slice(TILES // 8 * qq, TILES // 8 * (qq + 1))
                for k in range(2):
                    nc.sync.dma_start(xt_sb[:, k, qsl],
                                      XT[128 * k:128 * (k + 1), qsl])
                nc.gpsimd.dma_start(xb_sb[:, tsl, :], XBv[:, tsl, :])
                if qq == 0:
                    for k in range(2):
                        nc.scalar.dma_start(wm[:, k, :],
                                            WM[128 * k:128 * (k + 1), :])

            sp_insts = []
            spEv = spE[:].rearrange("p (t c) -> p c t", c=2)
            spv = sp[:].rearrange("p (t c) -> p c t", c=2)

            # ================= CNN + reg-broadcast phase =================
            with (
                tc.tile_pool(name="cnn", bufs=2) as cnnpool,
                tc.tile_pool(name="pc1", bufs=2, space="PSUM") as pc1pool,
                tc.tile_pool(name="pc2", bufs=1, space="PSUM") as pc2pool,
                tc.tile_pool(name="pc3", bufs=1, space="PSUM") as pc3pool,
                tc.tile_pool(name="pcb", bufs=2, space="PSUM") as pcbpool,
            ):
                p3 = pc3pool.tile([128, 2 * TILES], f32)
                for sg in range(NSG):
                    gs_range = range(GPS * sg, GPS * (sg + 1))
                    p2 = pc2pool.tile([128, 512], f32, tag="p2",
                                      name=f"p2sg{sg}")
                    for q, g in enumerate(gs_range):
                        sl = slice(512 * g, 512 * (g + 1))
                        p1 = pc1pool.tile([128, 512], f32, tag="p1",
                                          name=f"p1g{g}")
                        nc.tensor.matmul(p1[:], m1t[:, 0, :],
                                         xt_sb[:, 0, sl],
                                         start=True, stop=False)
                        nc.tensor.matmul(p1[:], m1t[:, 1, :],
                                         xt_sb[:, 1, sl],
                                         start=False, stop=True)
                        eh1 = cnnpool.tile([128, 512], f32, tag="eh1",
                                           name=f"eh1g{g}")
                        nc.scalar.activation(eh1[:], p1[:], AF.Exp,
                                             bias=b2v[:])
                        h1s = cnnpool.tile([128, 512], bf16, tag="h1s",
                                           name=f"h1sg{g}")
                        nc.scalar.activation(h1s[:], eh1[:], AF.Ln, bias=1.0)
                        nc.tensor.matmul(p2[32 * q:32 * (q + 1), :],
                                         m2bd[:], h1s[:],
                                         start=True, stop=True,
                                         tile_position=(0, 32 * q),
                                         skip_group_check=True)
                    eh2 = cnnpool.tile([128, 512], f32, tag="eh2",
                                       name=f"eh2sg{sg}")
                    nc.scalar.activation(eh2[:], p2[:], AF.Exp,
                                         bias=b3v[:])
                    h2s = cnnpool.tile([128, 512], bf16, tag="h2s",
                                       name=f"h2ssg{sg}")
                    nc.scalar.activation(h2s[:], eh2[:], AF.Ln, bias=1.0)
                    for q, g in enumerate(gs_range):
                        for i in range(4):
                            t = 4 * g + i
                            nc.tensor.matmul(
                                p3[:, 2 * t:2 * t + 2],
                                h2s[32 * q:32 * (q + 1),
                                    128 * i:128 * (i + 1)],
                                lwbd[32 * q:32 * (q + 1), :],
                                start=True, stop=True,
                                tile_position=(32 * q, 0),
                                skip_group_check=True)

                    sgt = slice(4 * GPS * sg, 4 * GPS * (sg + 1))
                    sgs = slice(8 * GPS * sg, 8 * GPS * (sg + 1))
                    nc.scalar.activation(spEv[:, 0, sgt],
                                         p3[:, sgs].rearrange(
                                             "p (t c) -> p c t", c=2)[:, 0, :],
                                         AF.Exp, bias=lbm[:])
                    nc.scalar.activation(spEv[:, 1, sgt],
                                         p3[:, sgs].rearrange(
                                             "p (t c) -> p c t", c=2)[:, 1, :],
                                         AF.Exp, bias=lbr[:])
                    sp_i = nc.scalar.activation(sp[:, sgs], spE[:, sgs],
                                                AF.Ln, bias=1.0)
                    sp_insts.append(sp_i)
                    nc.vector.tensor_scalar(gph[:, sgt], spv[:, 0, sgt],
                                            gsc[:], -0.25,
                                            Alu.mult, Alu.add)
                    nc.vector.tensor_scalar(c13x4[:, sgt], gph[:, sgt],
                                            8.0 / 3.0, 1.0,
                                            Alu.mult, Alu.add)
                    # bf16 copy of reg for the PE transpose
                    nc.vector.tensor_scalar(regc[:, sgt], spv[:, 1, sgt],
                                            1.0, None, Alu.mult)

                    # reg (this supergroup's half) -> regB broadcast along
                    # partitions, via PE transpose + 1-contract broadcast
                    # matmuls; then xr = reg (.) x for this batch half.
                    hsl = slice(BC // NSG * sg, BC // NSG * (sg + 1))
                    pT = pcbpool.tile([16, 128], f32, tag="pt",
                                      name=f"pT{sg}")
                    nc.tensor.matmul(pT[:], regc[:, sgt], eye[:],
                                     start=True, stop=True)
                    nc.vector.tensor_scalar(regT[sg][:], pT[:], 1.0, None,
                                            Alu.mult)
                    nc.sync.dma_start(
                        regf[:, hsl].rearrange("o (t n) -> o t n", n=128),
                        regT[sg][:].rearrange("t n -> t () n"))
                    for tt_ in range(TILES // NSG):
                        t = TILES // NSG * sg + tt_
                        bb = t // 4
                        pB = (pcbpool.tile([128, 512], f32, tag="pb",
                                           name=f"pB{bb}")
                              if t % 4 == 0 else pB)
                        nc.tensor.matmul(
                            pB[:, 128 * (t % 4):128 * (t % 4 + 1)],
                            onesr[:], regf[0:1, 128 * t:128 * (t + 1)],
                            start=(t % 4 == 0), stop=(t % 4 == 3),
                            skip_group_check=True)
                        if t % 4 == 3:
                            nc.vector.tensor_scalar(
                                regB[:, 512 * bb:512 * (bb + 1)], pB[:],
                                1.0, None, Alu.mult)
                    for k in range(2):
                        nc.vector.tensor_tensor(xr_sb[:, k, hsl],
                                                xt_sb[:, k, hsl],
                                                regB[:, hsl], Alu.mult)

            # ================= main phase =================
            s16_chunks = [None] * CHUNKS
            r_chunks = [None] * CHUNKS
            w_chunks = [None] * CHUNKS
            last_zd = None
            with tc.tile_pool(name="pm", bufs=3, space="PSUM") as pmpool:
                for c in range(CHUNKS):
                    pm = pmpool.tile([128, CF], f32, tag="pm", name=f"pm{c}")
                    # identity moves first (eye stationary; start=True resets
                    # each 512-col bank, so issue per-bank leader first)
                    for i in range(CT):
                        t = CT * c + i
                        nc.tensor.matmul(pm[:, 256 * i:256 * (i + 1)],
                                         eye[:], xb_sb[:, t, :],
                                         start=(i % 2 == 0), stop=False,
                                         skip_group_check=True)
                    for i in range(CT):
                        t = CT * c + i
                        tsl = slice(128 * t, 128 * (t + 1))
                        osl = slice(256 * i, 256 * (i + 1))
                        for k in range(2):
                            nc.tensor.matmul(
                                pm[:, osl], xt_sb[:, k, tsl],
                                wm[:, k, 0:256],
                                start=False, stop=False,
                                skip_group_check=True)
                            nc.tensor.matmul(
                                pm[:, osl], xr_sb[:, k, tsl],
                                wm[:, k, 256:512],
                                start=False, stop=(k == 1),
                                skip_group_check=True)
                    # E2 = (2p - 1/3)^2  [ACT, table-free]
                    e2 = e2pool.tile([128, CF], fp16, tag="e2",
                                     name=f"e2c{c}")
                    nc.scalar.activation(e2[:], pm[:], AF.Square,
                                         scale=2.0, bias=cm13[:])
                    # hm4 = E2 + c13x4 (per-tile scalar; fp16 4x mode)
                    hd = hdpool.tile([128, 2, CF], fp16, tag="hd",
                                     name=f"hd{c}")
                    for i in range(CT):
                        t = CT * c + i
                        osl = slice(256 * i, 256 * (i + 1))
                        nc.vector.tensor_scalar(hd[:, 0, osl], e2[:, osl],
                                                c13x4[:, t:t + 1], None,
                                                Alu.add)
                    # C8, D4
                    c8 = c8pool.tile([128, CF], fp16, tag="c8",
                                     name=f"c8c{c}")
                    nc.vector._custom_dve(
                        COPS['CH8'], out=c8[:], in0=hd[:, 0, :], in1=pm[:],
                        s0=1.0 / 3.0, s1=1.5, imm2=0.5)
                    nc.vector._custom_dve(
                        COPS['DQ'], out=hd[:, 1, :], in0=hd[:, 0, :],
                        in1=c8[:])
                    # z | irs = AbsRsqrt(hm4 | D4)  [one 2048-wide ACT op]
                    zd = rgpool.tile([128, 2, CF], fp16, tag="zd",
                                     name=f"zd{c}")
                    zd_i = nc.scalar.activation(
                        zd[:].rearrange("p a b -> p (a b)"),
                        hd[:].rearrange("p a b -> p (a b)"),
                        AF.Abs_reciprocal_sqrt, bias=epsv[:])
                    if last_zd is None:
                        for spi in sp_insts:
                            add_dep_helper(zd_i.ins, spi.ins, sync=True,
                                           reason="absrsqrt after NLE table")
                    else:
                        add_dep_helper(zd_i.ins, last_zd.ins, sync=False,
                                       reason="chain zd block")
                    last_zd = zd_i
                    # r = hm4*z,  w = C8*irs  (all-SBUF: Pool can take them)
                    r = rrpool.tile([128, CF], fp16, tag="rr", name=f"r{c}")
                    r_chunks[c] = r
                    nc.gpsimd.tensor_tensor(r[:], hd[:, 0, :], zd[:, 0, :],
                                            Alu.mult)
                    w = wvpool.tile([128, CF], fp16, tag="wv", name=f"w{c}")
                    w_chunks[c] = w
                    nc.vector.tensor_tensor(w[:], c8[:], zd[:, 1, :],
                                            Alu.mult)
                    # s16 = p + 1/3 (frees the PSUM bank pair; GPSIMD can't
                    # read PSUM, so split between DVE and ACT-Identity)
                    s16 = s16pool.tile([128, CF], fp16, tag="s16",
                                       name=f"s16c{c}")
                    s16_chunks[c] = s16
                    if c % 2 == 0:
                        nc.vector.tensor_scalar(s16[:], pm[:], 1.0 / 3.0,
                                                None, Alu.add)
                    else:
                        nc.scalar.activation(s16[:], pm[:], AF.Identity,
                                             bias=c13v[:])

            # ---- trig tail: arctan block, RGSIN, add, DMA out ----
            last_at = None
            for c in range(CHUNKS):
                w = w_chunks[c]
                at_i = nc.scalar.activation(w[:], w[:], AF.Arctan)
                if last_at is None:
                    add_dep_helper(at_i.ins, last_zd.ins, sync=True,
                                   reason="trig block after absrsqrt block")
                else:
                    add_dep_helper(at_i.ins, last_at.ins, sync=False,
                                   reason="chain trig block")
                last_at = at_i
                rg = rgpool.tile([128, CF], fp16, tag="rg", name=f"rgc{c}")
                nc.vector._custom_dve(
                    COPS['RGSIN'], out=rg[:], in0=w[:],
                    in1=r_chunks[c][:],
                    s0=-1.0 / 3.0, s1=1.0 / 162.0, imm2=-1.0 / 29160.0)
                ot = oopool.tile([128, CF], odt, tag="oo", name=f"o{c}")
                nc.vector.tensor_tensor(ot[:], rg[:], s16_chunks[c][:],
                                        Alu.add)
                dview = OUT[512 * c:512 * (c + 1), :].rearrange(
                    "(tt p) n -> p tt n", p=128)
                nc.sync.dma_start(
                    dview, ot[:].rearrange("p (tt n) -> p tt n", n=256))

    nc.compile()
    return nc


def _get_program():
    key = (B_TOTAL, N, N_CORES)
    if key not in _PROG:
        _PROG[key] = _build_program()
    return _PROG[key]


def _host_prep(inputs):
    import ml_dtypes
    bf = ml_dtypes.bfloat16
    x = _np_f32(inputs['x']).reshape(B_TOTAL, N)
    x_b = _np_f32(inputs['x_b']).reshape(B_TOTAL, N)
    m = float(np.asarray(inputs['mass']).reshape(-1)[0])
    gp = float(np.asarray(inputs['gamma_p']).reshape(-1)[0])
    gamma = float(np.log1p(np.exp(gp))) if gp < 30 else gp
    TtT = _np_f32(inputs['TtT'])
    DtD = _np_f32(inputs['DtD'])

    W_A = ((np.eye(N, dtype=np.float32) - np.float32(gamma) * TtT.T)
           / np.float32(3.0 * m)).astype(np.float32)
    W_B = (-np.float32(gamma) * DtD.T / np.float32(3.0 * m)).astype(np.float32)
    WM = np.concatenate([W_A, W_B], axis=1).astype(bf)          # (256,512)

    M1s, M2s, lws = {}, {}, {}
    for tag in ('mu', 'reg'):
        M1s[tag] = _conv_pool_mat(inputs['w2_' + tag], 256)      # (64,256)
        M2s[tag] = _conv_pool_mat(inputs['w3_' + tag], 64)       # (16,64)
        lws[tag] = _np_f32(inputs['lw_' + tag]).reshape(16)
    M1cat = np.concatenate([M1s['mu'], M1s['reg']], axis=0)      # (128,256)
    M1T = np.ascontiguousarray(M1cat.T).astype(bf)               # (256,128)
    M2BD = np.zeros((128, 32), np.float32)
    M2BD[0:64, 0:16] = M2s['mu'].T
    M2BD[64:128, 16:32] = M2s['reg'].T
    M2BD = M2BD.astype(bf)
    LWBD1 = np.zeros((32, 2), np.float32)
    LWBD1[0:16, 0] = lws['mu']
    LWBD1[16:32, 1] = lws['reg']
    LWBD = np.tile(LWBD1, (4, 1)).astype(bf)                     # (128,2)

    def sc(name):
        return float(np.asarray(inputs[name]).reshape(-1)[0])

    B2V = np.full((128, 1), sc('b2_mu'), np.float32)
    B2V[64:] = sc('b2_reg')
    B3V1 = np.full((32, 1), sc('b3_mu'), np.float32)
    B3V1[16:] = sc('b3_reg')
    B3V = np.tile(B3V1, (4, 1))                                  # (128,1)
    LBM = np.full((128, 1), sc('lb_mu'), np.float32)
    LBR = np.full((128, 1), sc('lb_reg'), np.float32)
    GSC = np.full((128, 1), gamma / (m * m), np.float32)

    EYEM = np.eye(128, dtype=np.float32).astype(bf)
    ONES = np.ones((1, 128), np.float32).astype(bf)
    consts = dict(wm=WM, m1t=M1T, m2bd=M2BD, lwbd=LWBD, eye=EYEM,
                  onesr=ONES, b2v=B2V, b3v=B3V, lbm=LBM, lbr=LBR, gsc=GSC)

    xb3 = (np.float32(gamma / (3.0 * m)) * x_b).astype(bf)
    xbf = x.astype(bf)
    in_maps = []
    for c in range(N_CORES):
        rows = slice(BC * c, BC * (c + 1))
        im = dict(consts)
        im['xt'] = np.ascontiguousarray(xbf[rows].T)
        im['xb'] = np.ascontiguousarray(xb3[rows])
        in_maps.append(im)
    return in_maps, m


def kernel(**inputs) -> np.ndarray:
    from concourse import bass_utils
    nc = _get_program()
    in_maps, m = _host_prep(inputs)
    res = bass_utils.run_bass_kernel_spmd(nc, in_maps,
                                          core_ids=list(range(N_CORES)))
    out = np.concatenate([res.results[c]['out'] for c in range(N_CORES)],
                         axis=0).astype(np.float32)
    if m != 1.0:
        out = (np.float32(m) * out).astype(np.float32)
    return np.ascontiguousarray(out.reshape(B_TOTAL, 1, N))


# revision 19
# speedup vs baseline: 1.1198x; 1.0384x over previous
"""Trainium2 Bass kernel for nn_Block_34067680592489.

Computes, for B=32768 independent signals x[b] (length 256):
  mu,reg = small-CNN(x[b])      (conv5+avgpool4+softplus twice, linear, softplus)
  grad   = TtT x - x_b + reg * DtD x
  x_t    = x - gamma * grad,  gamma = softplus(gamma_p)
  out    = middle root of z^3 -(m+x_t) z^2 + (m x_t - 2 gm) z + gm m,  gm = gamma*mu

Device algorithm (per element, normalized to mass=1; p = s - 1/3 lives in
PSUM directly as the matmul accumulation, s = (1+xt)/3):
  E   = 2p - 1/3  (= 2s - 1)
  E2  = E^2                                  (ACT Square of PSUM, table-free)
  hm4 = E2 + c13x4   (= 4(sqe + c13) = -4p/3... the cubic's -p coeff x4)
  C8  = E*(1.5*hm4 - 0.5*E^2 - 1.5)  (= 8C = -4q)    (fused DVE op)
  D4  = hm4^3 - C8^2 (= 64 D' > 0)                   (fused DVE op)
  z|irs = AbsRsqrt(hm4 | D4)       (ONE table for the whole mid-phase)
  r   = hm4 * z   (= 2 sqrt(hm))
  w   = C8 * irs  (= C/sqrt(D'))
  at  = Arctan(w)                            (trig table, tail block)
  root= s - r*sin(at/3)                      (DVE odd-poly * r, + s16)
The reg * DtD x term is folded into the PE contraction: xr = reg (.) x is
built once on DVE (bf16, 2x mode) from a PE-broadcast of the CNN's reg
output, so no per-tile scaling passes are needed; x_b enters PSUM via one
eye-stationary identity matmul per tile.

Sharding: pure data parallel over batch, 8 cores x 4096 rows.
"""

import numpy as np

B_TOTAL = 32768
N = 256
N_CORES = 8
BC = B_TOTAL // N_CORES      # rows per core
TILES = BC // 128            # 32 batch tiles of 128
CT = 4                       # tiles per elementwise chunk
CHUNKS = TILES // CT         # 8
CF = CT * N                  # chunk free size (1024)

_PROG = {}


def _np_f32(a):
    return np.ascontiguousarray(np.asarray(a, dtype=np.float32))


def _conv_pool_mat(w, L):
    """(L/4, L) matrix implementing conv1d(k=5,pad=2) then avgpool4."""
    taps = np.asarray(w, np.float32).reshape(5)
    C = np.zeros((L, L), np.float32)
    for n in range(L):
        for k in range(5):
            m = n + k - 2
            if 0 <= m < L:
                C[n, m] = taps[k]
    P = np.zeros((L // 4, L), np.float32)
    for i in range(L // 4):
        P[i, 4 * i:4 * i + 4] = 0.25
    return (P @ C).astype(np.float32)


_CUSTOM_OPS = {}


def _get_custom_ops():
    """Register this kernel's fused custom-DVE ops (idempotent).

    CH8:  C8 = (in0*s1 - E^2*imm2 - s1)*E,  E = 2*in1 - s0
          (called with in0=hm4, in1=p(PSUM), s0=1/3, s1=1.5, imm2=0.5)
    DQ:   D4 = in0^3 - in1^2                 (in0=hm4, in1=C8)
    RGSIN: rg = (at*(s0+at^2*(s1+at^2*imm2)))*r   (odd -sin(x/3) poly)
    """
    if _CUSTOM_OPS:
        return _CUSTOM_OPS
    import concourse.dve_ops as dops
    from concourse.dve_spec import (Spec, Src0, Src1, C0, C1, C2, sq,
                                    lower, _has_src1)
    from concourse.dve_uop import DveOpSpec

    def reg(name, spec):
        if name in dops._SUB_OPCODE_FOR_NAME:
            return next(o for o in dops.OPS if o.name == name)
        row = dops._CUSTOM_DVE_ROW_BASE + len(dops.OPS)
        assert row < 0x20
        dops._SUB_OPCODE_FOR_NAME[name] = row
        shas = {}
        for ver in ("v3", "v4"):
            u = lower(spec, ver=ver)
            shas[ver] = DveOpSpec(name=name, opcode=row, uops=u,
                                  rd1_en=_has_src1(spec)).sha(ver)
        op = dops.DveOp(name, spec, subdim=False, uops_sha=shas)
        dops.OPS.append(op)
        dops.CUSTOM_DVE_SPECS[name] = spec
        return op

    import numpy as np_

    _E = (Src1 + Src1) - C0
    _CUSTOM_OPS['CH8'] = reg('ANT_K_CH8', Spec(
        body=((Src0 * C1) - sq(_E) * C2 - C1) * _E,
        reference=lambda in0, in1, s0, s1, imm2:
            (((in0 * s1) - (2.0 * in1 - s0) ** 2 * imm2 - s1)
             * (2.0 * in1 - s0)).astype(np_.float32),
    ))
    _CUSTOM_OPS['DQ'] = reg('ANT_K_DQ', Spec(
        body=(sq(Src0) * Src0) - sq(Src1),
        reference=lambda in0, in1, s0, s1, imm2:
            (in0 ** 2 * in0 - in1 ** 2).astype(np_.float32),
    ))
    _a2 = sq(Src0)
    _CUSTOM_OPS['RGSIN'] = reg('ANT_K_RGSIN', Spec(
        body=(Src0 * (C0 + _a2 * (C1 + _a2 * C2))) * Src1,
        reference=lambda in0, in1, s0, s1, imm2:
            ((in0 * (s0 + in0 * in0 * (s1 + in0 * in0 * imm2))) * in1
             ).astype(np_.float32),
    ))
    return _CUSTOM_OPS


_TABLES_PATCHED = False


def _patch_act_tables():
    """Restrict ACT table-set choice to the sets this kernel uses."""
    global _TABLES_PATCHED
    if _TABLES_PATCHED:
        return
    import concourse.bacc as bacc
    keep = {'natural_log_exp_and_others',
            'abs_reciprocal_sqrt_and_small', 'trig_and_small'}
    orig = bacc.get_activation_tables

    def patched(arch):
        t = orig(arch)
        return {k: (v if k in keep else set()) for k, v in t.items()}

    bacc.get_activation_tables = patched
    _TABLES_PATCHED = True


def _build_program():
    import concourse.bacc as bacc
    import concourse.tile as tile
    import concourse.mybir as mybir
    from concourse.tile import add_dep_helper
    _patch_act_tables()

    dt = mybir.dt
    f32 = dt.float32
    bf16 = dt.bfloat16
    fp16 = dt.float16
    Alu = mybir.AluOpType
    AF = mybir.ActivationFunctionType
    odt = bf16

    COPS = _get_custom_ops()
    nc = bacc.Bacc("TRN2", target_bir_lowering=False, debug=False,
                   num_devices=N_CORES)

    XT = nc.dram_tensor("xt", (256, BC), bf16, kind="ExternalInput")
    XB = nc.dram_tensor("xb", (BC, 256), bf16, kind="ExternalInput")
    EYE = nc.dram_tensor("eye", (128, 128), bf16, kind="ExternalInput")
    WM = nc.dram_tensor("wm", (256, 512), bf16, kind="ExternalInput")
    M1T = nc.dram_tensor("m1t", (256, 128), bf16, kind="ExternalInput")
    M2BD = nc.dram_tensor("m2bd", (128, 32), bf16, kind="ExternalInput")
    LWBD = nc.dram_tensor("lwbd", (128, 2), bf16, kind="ExternalInput")
    B2V = nc.dram_tensor("b2v", (128, 1), f32, kind="ExternalInput")
    B3V = nc.dram_tensor("b3v", (128, 1), f32, kind="ExternalInput")
    LBM = nc.dram_tensor("lbm", (128, 1), f32, kind="ExternalInput")
    LBR = nc.dram_tensor("lbr", (128, 1), f32, kind="ExternalInput")
    GSC = nc.dram_tensor("gsc", (128, 1), f32, kind="ExternalInput")
    ONESR = nc.dram_tensor("onesr", (1, 128), bf16, kind="ExternalInput")
    OUT = nc.dram_tensor("out", (BC, 256), odt, kind="ExternalOutput")

    NSG = 2                          # supergroups
    GPS = CHUNKS // NSG              # groups per supergroup

    with tile.TileContext(nc) as tc:
        with (
            tc.tile_pool(name="const", bufs=1) as cpool,
            tc.tile_pool(name="xin", bufs=1) as xtpool,
            tc.tile_pool(name="hd", bufs=3) as hdpool,
            tc.tile_pool(name="c8", bufs=3) as c8pool,
            tc.tile_pool(name="e2", bufs=2) as e2pool,
            tc.tile_pool(name="s16", bufs=CHUNKS) as s16pool,
            tc.tile_pool(name="rr", bufs=CHUNKS) as rrpool,
            tc.tile_pool(name="wv", bufs=CHUNKS) as wvpool,
            tc.tile_pool(name="rg", bufs=3) as rgpool,
            tc.tile_pool(name="oo", bufs=3) as oopool,
        ):
            # ---- constants into SBUF ----
            wm = cpool.tile([128, 2, 512], bf16)
            m1t = cpool.tile([128, 2, 128], bf16)
            m2bd = cpool.tile([128, 32], bf16)
            lwbd = cpool.tile([128, 2], bf16)
            b2v = cpool.tile([128, 1], f32)
            b3v = cpool.tile([128, 1], f32)
            lbm = cpool.tile([128, 1], f32)
            lbr = cpool.tile([128, 1], f32)
            gsc = cpool.tile([128, 1], f32)
            spE = cpool.tile([128, 2 * TILES], f32)
            sp = cpool.tile([128, 2 * TILES], f32)
            gph = cpool.tile([128, TILES], f32)
            c13x4 = cpool.tile([128, TILES], f32)
            eye = cpool.tile([128, 128], bf16)
            onesr = cpool.tile([1, 128], bf16)
            regc = cpool.tile([128, TILES], bf16)
            regT = [cpool.tile([16, 128], bf16, name=f"regT{i}")
                    for i in range(NSG)]
            regf = cpool.tile([1, BC], bf16)
            regB = cpool.tile([128, BC], bf16)
            epsv = cpool.tile([128, 1], f32)
            nc.vector.memset(epsv[:], 1e-5)
            cm13 = cpool.tile([128, 1], f32)
            nc.vector.memset(cm13[:], -1.0 / 3.0)
            c13v = cpool.tile([128, 1], f32)
            nc.vector.memset(c13v[:], 1.0 / 3.0)
            nc.sync.dma_start(eye[:], EYE[:])
            nc.sync.dma_start(onesr[:], ONESR[:])
            for k in range(2):
                nc.sync.dma_start(m1t[:, k, :], M1T[128 * k:128 * (k + 1), :])
            nc.sync.dma_start(m2bd[:], M2BD[:])
            nc.sync.dma_start(lwbd[:], LWBD[:])
            nc.sync.dma_start(b2v[:], B2V[:])
            nc.sync.dma_start(b3v[:], B3V[:])
            nc.sync.dma_start(lbm[:], LBM[:])
            nc.sync.dma_start(lbr[:], LBR[:])
            nc.sync.dma_start(gsc[:], GSC[:])

            # ---- inputs ----
            xt_sb = xtpool.tile([128, 2, BC], bf16)
            xr_sb = xtpool.tile([128, 2, BC], bf16)
            xb_sb = xtpool.tile([128, TILES, 256], bf16)
            XBv = XB[:].rearrange("(t p) n -> p t n", p=128)
            for qq in range(8):
                qsl = slice(BC // 8 * qq, BC // 8 * (qq + 1))
                tsl = slice(TILES // 8 * qq, TILES // 8 * (qq + 1))
                for k in range(2):
                    nc.sync.dma_start(xt_sb[:, k, qsl],
                                      XT[128 * k:128 * (k + 1), qsl])
                nc.gpsimd.dma_start(xb_sb[:, tsl, :], XBv[:, tsl, :])
                if qq == 0:
                    for k in range(2):
                        nc.scalar.dma_start(wm[:, k, :],
                                            WM[128 * k:128 * (k + 1), :])

            sp_insts = []
            spEv = spE[:].rearrange("p (t c) -> p c t", c=2)
            spv = sp[:].rearrange("p (t c) -> p c t", c=2)

            # ================= CNN + reg-broadcast phase =================
            with (
                tc.tile_pool(name="cnn", bufs=2) as cnnpool,
                tc.tile_pool(name="pc1", bufs=2, space="PSUM") as pc1pool,
                tc.tile_pool(name="pc2", bufs=1, space="PSUM") as pc2pool,
                tc.tile_pool(name="pc3", bufs=1, space="PSUM") as pc3pool,
                tc.tile_pool(name="pcb", bufs=2, space="PSUM") as pcbpool,
            ):
                p3 = pc3pool.tile([128, 2 * TILES], f32)
                for sg in range(NSG):
                    gs_range = range(GPS * sg, GPS * (sg + 1))
                    p2 = pc2pool.tile([128, 512], f32, tag="p2",
                                      name=f"p2sg{sg}")
                    for q, g in enumerate(gs_range):
                        sl = slice(512 * g, 512 * (g + 1))
                        p1 = pc1pool.tile([128, 512], f32, tag="p1",
                                          name=f"p1g{g}")
                        nc.tensor.matmul(p1[:], m1t[:, 0, :],
                                         xt_sb[:, 0, sl],
                                         start=True, stop=False)
                        nc.tensor.matmul(p1[:], m1t[:, 1, :],
                                         xt_sb[:, 1, sl],
                                         start=False, stop=True)
                        eh1 = cnnpool.tile([128, 512], f32, tag="eh1",
                                           name=f"eh1g{g}")
                        nc.scalar.activation(eh1[:], p1[:], AF.Exp,
                                             bias=b2v[:])
                        h1s = cnnpool.tile([128, 512], bf16, tag="h1s",
                                           name=f"h1sg{g}")
                        nc.scalar.activation(h1s[:], eh1[:], AF.Ln, bias=1.0)
                        nc.tensor.matmul(p2[32 * q:32 * (q + 1), :],
                                         m2bd[:], h1s[:],
                                         start=True, stop=True,
                                         tile_position=(0, 32 * q),
                                         skip_group_check=True)
                    eh2 = cnnpool.tile([128, 512], f32, tag="eh2",
                                       name=f"eh2sg{sg}")
                    nc.scalar.activation(eh2[:], p2[:], AF.Exp,
                                         bias=b3v[:])
                    h2s = cnnpool.tile([128, 512], bf16, tag="h2s",
                                       name=f"h2ssg{sg}")
                    nc.scalar.activation(h2s[:], eh2[:], AF.Ln, bias=1.0)
                    for q, g in enumerate(gs_range):
                        for i in range(4):
                            t = 4 * g + i
                            nc.tensor.matmul(
                                p3[:, 2 * t:2 * t + 2],
                                h2s[32 * q:32 * (q + 1),
                                    128 * i:128 * (i + 1)],
                                lwbd[32 * q:32 * (q + 1), :],
                                start=True, stop=True,
                                tile_position=(32 * q, 0),
                                skip_group_check=True)

                    sgt = slice(4 * GPS * sg, 4 * GPS * (sg + 1))
                    sgs = slice(8 * GPS * sg, 8 * GPS * (sg + 1))
                    nc.scalar.activation(spEv[:, 0, sgt],
                                         p3[:, sgs].rearrange(
                                             "p (t c) -> p c t", c=2)[:, 0, :],
                                         AF.Exp, bias=lbm[:])
                    nc.scalar.activation(spEv[:, 1, sgt],
                                         p3[:, sgs].rearrange(
                                             "p (t c) -> p c t", c=2)[:, 1, :],
                                         AF.Exp, bias=lbr[:])
                    sp_i = nc.scalar.activation(sp[:, sgs], spE[:, sgs],
                                                AF.Ln, bias=1.0)
                    sp_insts.append(sp_i)
                    nc.vector.tensor_scalar(gph[:, sgt], spv[:, 0, sgt],
                                            gsc[:], -0.25,
                                            Alu.mult, Alu.add)
                    nc.vector.tensor_scalar(c13x4[:, sgt], gph[:, sgt],
                                            8.0 / 3.0, 1.0,
                                            Alu.mult, Alu.add)
                    # bf16 copy of reg for the PE transpose
                    nc.vector.tensor_scalar(regc[:, sgt], spv[:, 1, sgt],
                                            1.0, None, Alu.mult)

                    # reg (this supergroup's half) -> regB broadcast along
                    # partitions, via PE transpose + 1-contract broadcast
                    # matmuls; then xr = reg (.) x for this batch half.
                    hsl = slice(BC // NSG * sg, BC // NSG * (sg + 1))
                    pT = pcbpool.tile([16, 128], f32, tag="pt",
                                      name=f"pT{sg}")
                    nc.tensor.matmul(pT[:], regc[:, sgt], eye[:],
                                     start=True, stop=True)
                    nc.vector.tensor_scalar(regT[sg][:], pT[:], 1.0, None,
                                            Alu.mult)
                    nc.sync.dma_start(
                        regf[:, hsl].rearrange("o (t n) -> o t n", n=128),
                        regT[sg][:].rearrange("t n -> t () n"))
                    for tt_ in range(TILES // NSG):
                        t = TILES // NSG * sg + tt_
                        bb = t // 4
                        pB = (pcbpool.tile([128, 512], f32, tag="pb",
                                           name=f"pB{bb}")
                              if t % 4 == 0 else pB)
                        nc.tensor.matmul(
                            pB[:, 128 * (t % 4):128 * (t % 4 + 1)],
                            onesr[:], regf[0:1, 128 * t:128 * (t + 1)],
                            start=(t % 4 == 0), stop=(t % 4 == 3),
                            skip_group_check=True)
                        if t % 4 == 3:
                            nc.vector.tensor_scalar(
                                regB[:, 512 * bb:512 * (bb + 1)], pB[:],
                                1.0, None, Alu.mult)
                    for k in range(2):
                        nc.vector.tensor_tensor(xr_sb[:, k, hsl],
                                                xt_sb[:, k, hsl],
                                                regB[:, hsl], Alu.mult)

            # ================= main phase =================
            s16_chunks = [None] * CHUNKS
            r_chunks = [None] * CHUNKS
            w_chunks = [None] * CHUNKS
            last_zd = None
            with tc.tile_pool(name="pm", bufs=3, space="PSUM") as pmpool:
                for c in range(CHUNKS):
                    pm = pmpool.tile([128, CF], f32, tag="pm", name=f"pm{c}")
                    # identity moves first (eye stationary; start=True resets
                    # each 512-col bank, so issue per-bank leader first)
                    for i in range(CT):
                        t = CT * c + i
                        nc.tensor.matmul(pm[:, 256 * i:256 * (i + 1)],
                                         eye[:], xb_sb[:, t, :],
                                         start=(i % 2 == 0), stop=False,
                                         skip_group_check=True)
                    for i in range(CT):
                        t = CT * c + i
                        tsl = slice(128 * t, 128 * (t + 1))
                        osl = slice(256 * i, 256 * (i + 1))
                        for k in range(2):
                            nc.tensor.matmul(
                                pm[:, osl], xt_sb[:, k, tsl],
                                wm[:, k, 0:256],
                                start=False, stop=False,
                                skip_group_check=True)
                            nc.tensor.matmul(
                                pm[:, osl], xr_sb[:, k, tsl],
                                wm[:, k, 256:512],
                                start=False, stop=(k == 1),
                                skip_group_check=True)
                    # E2 = (2p - 1/3)^2  [ACT, table-free]
                    e2 = e2pool.tile([128, CF], fp16, tag="e2",
                                     name=f"e2c{c}")
                    nc.scalar.activation(e2[:], pm[:], AF.Square,
                                         scale=2.0, bias=cm13[:])
                    # hm4 = E2 + c13x4 (per-tile scalar; fp16 4x mode)
                    hd = hdpool.tile([128, 2, CF], fp16, tag="hd",
                                     name=f"hd{c}")
                    for i in range(CT):
                        t = CT * c + i
                        osl = slice(256 * i, 256 * (i + 1))
                        nc.vector.tensor_scalar(hd[:, 0, osl], e2[:, osl],
                                                c13x4[:, t:t + 1], None,
                                                Alu.add)
                    # C8, D4
                    c8 = c8pool.tile([128, CF], fp16, tag="c8",
                                     name=f"c8c{c}")
                    nc.vector._custom_dve(
                        COPS['CH8'], out=c8[:], in0=hd[:, 0, :], in1=pm[:],
                        s0=1.0 / 3.0, s1=1.5, imm2=0.5)
                    nc.vector._custom_dve(
                        COPS['DQ'], out=hd[:, 1, :], in0=hd[:, 0, :],
                        in1=c8[:])
                    # z | irs = AbsRsqrt(hm4 | D4)  [one 2048-wide ACT op]
                    zd = rgpool.tile([128, 2, CF], fp16, tag="zd",
                                     name=f"zd{c}")
                    zd_i = nc.scalar.activation(
                        zd[:].rearrange("p a b -> p (a b)"),
                        hd[:].rearrange("p a b -> p (a b)"),
                        AF.Abs_reciprocal_sqrt, bias=epsv[:])
                    if last_zd is None:
                        for spi in sp_insts:
                            add_dep_helper(zd_i.ins, spi.ins, sync=True,
                                           reason="absrsqrt after NLE table")
                    else:
                        add_dep_helper(zd_i.ins, last_zd.ins, sync=False,
                                       reason="chain zd block")
                    last_zd = zd_i
                    # r = hm4*z,  w = C8*irs  (all-SBUF: Pool can take them)
                    r = rrpool.tile([128, CF], fp16, tag="rr", name=f"r{c}")
                    r_chunks[c] = r
                    nc.vector.tensor_tensor(r[:], hd[:, 0, :], zd[:, 0, :],
                                            Alu.mult)
                    w = wvpool.tile([128, CF], fp16, tag="wv", name=f"w{c}")
                    w_chunks[c] = w
                    nc.vector.tensor_tensor(w[:], c8[:], zd[:, 1, :],
                                            Alu.mult)
                    # s16 = p + 1/3 (frees the PSUM bank pair; GPSIMD can't
                    # read PSUM, so split between DVE and ACT-Identity)
                    s16 = s16pool.tile([128, CF], fp16, tag="s16",
                                       name=f"s16c{c}")
                    s16_chunks[c] = s16
                    if c % 2 == 0:
                        nc.vector.tensor_scalar(s16[:], pm[:], 1.0 / 3.0,
                                                None, Alu.add)
                    else:
                        nc.scalar.activation(s16[:], pm[:], AF.Identity,
                                             bias=c13v[:])

            # ---- trig tail: arctan block, RGSIN, add, DMA out ----
            last_at = None
            for c in range(CHUNKS):
                w = w_chunks[c]
                at_i = nc.scalar.activation(w[:], w[:], AF.Arctan)
                if last_at is None:
                    add_dep_helper(at_i.ins, last_zd.ins, sync=True,
                                   reason="trig block after absrsqrt block")
                else:
                    add_dep_helper(at_i.ins, last_at.ins, sync=False,
                                   reason="chain trig block")
                last_at = at_i
                rg = rgpool.tile([128, CF], fp16, tag="rg", name=f"rgc{c}")
                nc.vector._custom_dve(
                    COPS['RGSIN'], out=rg[:], in0=w[:],
                    in1=r_chunks[c][:],
                    s0=-1.0 / 3.0, s1=1.0 / 162.0, imm2=-1.0 / 29160.0)
                ot = oopool.tile([128, CF], odt, tag="oo", name=f"o{c}")
                nc.vector.tensor_tensor(ot[:], rg[:], s16_chunks[c][:],
                                        Alu.add)
                dview = OUT[512 * c:512 * (c + 1), :].rearrange(
                    "(tt p) n -> p tt n", p=128)
                nc.sync.dma_start(
                    dview, ot[:].rearrange("p (tt n) -> p tt n", n=256))

    nc.compile()
    return nc


def _get_program():
    key = (B_TOTAL, N, N_CORES)
    if key not in _PROG:
        _PROG[key] = _build_program()
    return _PROG[key]


def _host_prep(inputs):
    import ml_dtypes
    bf = ml_dtypes.bfloat16
    x = _np_f32(inputs['x']).reshape(B_TOTAL, N)
    x_b = _np_f32(inputs['x_b']).reshape(B_TOTAL, N)
    m = float(np.asarray(inputs['mass']).reshape(-1)[0])
    gp = float(np.asarray(inputs['gamma_p']).reshape(-1)[0])
    gamma = float(np.log1p(np.exp(gp))) if gp < 30 else gp
    TtT = _np_f32(inputs['TtT'])
    DtD = _np_f32(inputs['DtD'])

    W_A = ((np.eye(N, dtype=np.float32) - np.float32(gamma) * TtT.T)
           / np.float32(3.0 * m)).astype(np.float32)
    W_B = (-np.float32(gamma) * DtD.T / np.float32(3.0 * m)).astype(np.float32)
    WM = np.concatenate([W_A, W_B], axis=1).astype(bf)          # (256,512)

    M1s, M2s, lws = {}, {}, {}
    for tag in ('mu', 'reg'):
        M1s[tag] = _conv_pool_mat(inputs['w2_' + tag], 256)      # (64,256)
        M2s[tag] = _conv_pool_mat(inputs['w3_' + tag], 64)       # (16,64)
        lws[tag] = _np_f32(inputs['lw_' + tag]).reshape(16)
    M1cat = np.concatenate([M1s['mu'], M1s['reg']], axis=0)      # (128,256)
    M1T = np.ascontiguousarray(M1cat.T).astype(bf)               # (256,128)
    M2BD = np.zeros((128, 32), np.float32)
    M2BD[0:64, 0:16] = M2s['mu'].T
    M2BD[64:128, 16:32] = M2s['reg'].T
    M2BD = M2BD.astype(bf)
    LWBD1 = np.zeros((32, 2), np.float32)
    LWBD1[0:16, 0] = lws['mu']
    LWBD1[16:32, 1] = lws['reg']
    LWBD = np.tile(LWBD1, (4, 1)).astype(bf)                     # (128,2)

    def sc(name):
        return float(np.asarray(inputs[name]).reshape(-1)[0])

    B2V = np.full((128, 1), sc('b2_mu'), np.float32)
    B2V[64:] = sc('b2_reg')
    B3V1 = np.full((32, 1), sc('b3_mu'), np.float32)
    B3V1[16:] = sc('b3_reg')
    B3V = np.tile(B3V1, (4, 1))                                  # (128,1)
    LBM = np.full((128, 1), sc('lb_mu'), np.float32)
    LBR = np.full((128, 1), sc('lb_reg'), np.float32)
    GSC = np.full((128, 1), gamma / (m * m), np.float32)

    EYEM = np.eye(128, dtype=np.float32).astype(bf)
    ONES = np.ones((1, 128), np.float32).astype(bf)
    consts = dict(wm=WM, m1t=M1T, m2bd=M2BD, lwbd=LWBD, eye=EYEM,
                  onesr=ONES, b2v=B2V, b3v=B3V, lbm=LBM, lbr=LBR, gsc=GSC)

    xb3 = (np.float32(gamma / (3.0 * m)) * x_b).astype(bf)
    xbf = x.astype(bf)
    in_maps = []
    for c in range(N_CORES):
        rows = slice(BC * c, BC * (c + 1))
        im = dict(consts)
        im['xt'] = np.ascontiguousarray(xbf[rows].T)
        im['xb'] = np.ascontiguousarray(xb3[rows])
        in_maps.append(im)
    return in_maps, m


def kernel(**inputs) -> np.ndarray:
    from concourse import bass_utils
    nc = _get_program()
    in_maps, m = _host_prep(inputs)
    res = bass_utils.run_bass_kernel_spmd(nc, in_maps,
                                          core_ids=list(range(N_CORES)))
    out = np.concatenate([res.results[c]['out'] for c in range(N_CORES)],
                         axis=0).astype(np.float32)
    if m != 1.0:
        out = (np.float32(m) * out).astype(np.float32)
    return np.ascontiguousarray(out.reshape(B_TOTAL, 1, N))


# revision 84
# speedup vs baseline: 1.3628x; 1.2170x over previous
"""Trainium2 Bass kernel for nn_Block_34067680592489.

Computes, for B=32768 independent signals x[b] (length 256):
  mu,reg = small-CNN(x[b])      (conv5+avgpool4+softplus twice, linear, softplus)
  grad   = TtT x - x_b + reg * DtD x
  x_t    = x - gamma * grad,  gamma = softplus(gamma_p)
  out    = middle root of z^3 -(m+x_t) z^2 + (m x_t - 2 gm) z + gm m,  gm = gamma*mu

Device algorithm (per element, normalized to mass=1; p = s - 1/3 lives in
PSUM directly as the matmul accumulation, s = (1+xt)/3):
  E   = 2p - 1/3  (= 2s - 1)
  E2  = E^2                                  (ACT Square of PSUM, table-free)
  hm4 = E2 + c13x4   (= 4(sqe + c13) = -4p/3... the cubic's -p coeff x4)
  C8  = E*(1.5*hm4 - 0.5*E^2 - 1.5)  (= 8C = -4q)    (fused DVE op)
  D4  = hm4^3 - C8^2 (= 64 D' > 0)                   (fused DVE op)
  z|irs = AbsRsqrt(hm4 | D4)       (ONE table for the whole mid-phase)
  r   = hm4 * z   (= 2 sqrt(hm))
  w   = C8 * irs  (= C/sqrt(D'))
  at  = Arctan(w)                            (trig table, tail block)
  root= s - r*sin(at/3)                      (DVE odd-poly * r, + s16)
The reg * DtD x term is folded into the PE contraction: xr = reg (.) x is
built once on DVE (bf16, 2x mode) from a PE-broadcast of the CNN's reg
output, so no per-tile scaling passes are needed; x_b enters PSUM via one
eye-stationary identity matmul per tile.

Sharding: pure data parallel over batch, 8 cores x 4096 rows.
"""

import numpy as np

B_TOTAL = 32768
N = 256
N_CORES = 8
BC = B_TOTAL // N_CORES      # rows per core
TILES = BC // 128            # 32 batch tiles of 128
CT = 4                       # tiles per elementwise chunk
CHUNKS = TILES // CT         # 8
CF = CT * N                  # chunk free size (1024)

_PROG = {}


def _np_f32(a):
    return np.ascontiguousarray(np.asarray(a, dtype=np.float32))


def _conv_pool_mat(w, L):
    """(L/4, L) matrix implementing conv1d(k=5,pad=2) then avgpool4."""
    taps = np.asarray(w, np.float32).reshape(5)
    C = np.zeros((L, L), np.float32)
    for n in range(L):
        for k in range(5):
            m = n + k - 2
            if 0 <= m < L:
                C[n, m] = taps[k]
    P = np.zeros((L // 4, L), np.float32)
    for i in range(L // 4):
        P[i, 4 * i:4 * i + 4] = 0.25
    return (P @ C).astype(np.float32)


_CUSTOM_OPS = {}


def _get_custom_ops():
    """Register this kernel's fused custom-DVE ops (idempotent).

    CH8:  C8 = (in0*s1 - E^2*imm2 - s1)*E,  E = 2*in1 - s0
          (called with in0=hm4, in1=p(PSUM), s0=1/3, s1=1.5, imm2=0.5)
    DQ:   D4 = in0^3 - in1^2                 (in0=hm4, in1=C8)
    RGSIN: rg = (at*(s0+at^2*(s1+at^2*imm2)))*r   (odd -sin(x/3) poly)
    """
    if _CUSTOM_OPS:
        return _CUSTOM_OPS
    import concourse.dve_ops as dops
    from concourse.dve_spec import (Spec, Src0, Src1, C0, C1, C2, sq,
                                    lower, _has_src1)
    from concourse.dve_uop import DveOpSpec

    def reg(name, spec):
        if name in dops._SUB_OPCODE_FOR_NAME:
            return next(o for o in dops.OPS if o.name == name)
        row = dops._CUSTOM_DVE_ROW_BASE + len(dops.OPS)
        assert row < 0x20
        dops._SUB_OPCODE_FOR_NAME[name] = row
        shas = {}
        for ver in ("v3", "v4"):
            u = lower(spec, ver=ver)
            shas[ver] = DveOpSpec(name=name, opcode=row, uops=u,
                                  rd1_en=_has_src1(spec)).sha(ver)
        op = dops.DveOp(name, spec, subdim=False, uops_sha=shas)
        dops.OPS.append(op)
        dops.CUSTOM_DVE_SPECS[name] = spec
        return op

    import numpy as np_

    _E = (Src1 + Src1) - C0
    _CUSTOM_OPS['CH8'] = reg('ANT_K_CH8', Spec(
        body=((Src0 * C1) - sq(_E) * C2 - C1) * _E,
        reference=lambda in0, in1, s0, s1, imm2:
            (((in0 * s1) - (2.0 * in1 - s0) ** 2 * imm2 - s1)
             * (2.0 * in1 - s0)).astype(np_.float32),
    ))
    _CUSTOM_OPS['DQ'] = reg('ANT_K_DQ', Spec(
        body=(sq(Src0) * Src0) - sq(Src1),
        reference=lambda in0, in1, s0, s1, imm2:
            (in0 ** 2 * in0 - in1 ** 2).astype(np_.float32),
    ))
    _a2 = sq(Src0)
    _CUSTOM_OPS['RGSIN'] = reg('ANT_K_RGSIN', Spec(
        body=(Src0 * (C0 + _a2 * (C1 + _a2 * C2))) * Src1,
        reference=lambda in0, in1, s0, s1, imm2:
            ((in0 * (s0 + in0 * in0 * (s1 + in0 * in0 * imm2))) * in1
             ).astype(np_.float32),
    ))
    return _CUSTOM_OPS


_TABLES_PATCHED = False


def _patch_act_tables():
    """Restrict ACT table-set choice to the sets this kernel uses."""
    global _TABLES_PATCHED
    if _TABLES_PATCHED:
        return
    import concourse.bacc as bacc
    keep = {'natural_log_exp_and_others',
            'abs_reciprocal_sqrt_and_small', 'trig_and_small'}
    orig = bacc.get_activation_tables

    def patched(arch):
        t = orig(arch)
        return {k: (v if k in keep else set()) for k, v in t.items()}

    bacc.get_activation_tables = patched
    _TABLES_PATCHED = True


def _build_program():
    import concourse.bacc as bacc
    import concourse.tile as tile
    import concourse.mybir as mybir
    from concourse.tile import add_dep_helper
    _patch_act_tables()

    dt = mybir.dt
    f32 = dt.float32
    bf16 = dt.bfloat16
    fp16 = dt.float16
    Alu = mybir.AluOpType
    AF = mybir.ActivationFunctionType
    odt = bf16

    COPS = _get_custom_ops()
    nc = bacc.Bacc("TRN2", target_bir_lowering=False, debug=False,
                   num_devices=N_CORES)

    XT = nc.dram_tensor("xt", (256, BC), bf16, kind="ExternalInput")
    XB = nc.dram_tensor("xb", (BC, 256), bf16, kind="ExternalInput")
    EYE = nc.dram_tensor("eye", (128, 128), bf16, kind="ExternalInput")
    WM = nc.dram_tensor("wm", (256, 512), bf16, kind="ExternalInput")
    M1T = nc.dram_tensor("m1t", (256, 128), bf16, kind="ExternalInput")
    M2BD = nc.dram_tensor("m2bd", (128, 32), bf16, kind="ExternalInput")
    LWBD = nc.dram_tensor("lwbd", (128, 2), bf16, kind="ExternalInput")
    B2V = nc.dram_tensor("b2v", (128, 1), f32, kind="ExternalInput")
    B3V = nc.dram_tensor("b3v", (128, 1), f32, kind="ExternalInput")
    LBM = nc.dram_tensor("lbm", (128, 1), f32, kind="ExternalInput")
    LBR = nc.dram_tensor("lbr", (128, 1), f32, kind="ExternalInput")
    GSC = nc.dram_tensor("gsc", (128, 1), f32, kind="ExternalInput")
    ONESR = nc.dram_tensor("onesr", (1, 128), bf16, kind="ExternalInput")
    REGD = nc.dram_tensor("regd", (BC,), bf16, kind="Internal")
    OUT = nc.dram_tensor("out", (128, TILES * 256), odt,
                         kind="ExternalOutput")

    NSG = 2                          # supergroups
    GPS = CHUNKS // NSG              # groups per supergroup

    with tile.TileContext(nc) as tc:
        with (
            tc.tile_pool(name="const", bufs=1) as cpool,
            tc.tile_pool(name="xin", bufs=1) as xtpool,
            tc.tile_pool(name="hd", bufs=5) as hdpool,
            tc.tile_pool(name="c8", bufs=5) as c8pool,
            tc.tile_pool(name="e2", bufs=2) as e2pool,
            tc.tile_pool(name="s16", bufs=CHUNKS) as s16pool,
            tc.tile_pool(name="rr", bufs=CHUNKS) as rrpool,
            tc.tile_pool(name="wv", bufs=CHUNKS) as wvpool,
            tc.tile_pool(name="rg", bufs=5) as rgpool,
            tc.tile_pool(name="oo", bufs=3) as oopool,
            tc.tile_pool(name="pcb", bufs=1, space="PSUM") as pcbpool,
        ):
            # ---- constants into SBUF ----
            wm = cpool.tile([128, 2, 512], bf16)
            m1t = cpool.tile([128, 2, 128], bf16)
            m2bd = cpool.tile([128, 32], bf16)
            lwbd = cpool.tile([128, 2], bf16)
            b2v = cpool.tile([128, 1], f32)
            b3v = cpool.tile([128, 1], f32)
            lbm = cpool.tile([128, 1], f32)
            lbr = cpool.tile([128, 1], f32)
            gsc = cpool.tile([128, 1], f32)
            spE = cpool.tile([128, 2 * TILES], f32)
            sp = cpool.tile([128, 2 * TILES], f32)
            gph = cpool.tile([128, TILES], f32)
            c13x4 = cpool.tile([128, TILES], f32)
            eye = cpool.tile([128, 128], bf16)
            onesr = cpool.tile([1, 128], bf16)
            regc = cpool.tile([128, TILES], bf16)
            regf = cpool.tile([1, BC], bf16)
            epsv = cpool.tile([128, 1], f32)
            nc.vector.memset(epsv[:], 1e-5)
            cm13 = cpool.tile([128, 1], f32)
            nc.vector.memset(cm13[:], -1.0 / 3.0)
            c13v = cpool.tile([128, 1], f32)
            nc.vector.memset(c13v[:], 1.0 / 3.0)
            # ---- inputs: one tile per slice so readers only wait on the
            # DMAs they actually need (whole-tile deps otherwise stall the
            # first matmul until every input byte has landed) ----
            xts = [xtpool.tile([128, 2, 512], bf16, name=f"xts{g}")
                   for g in range(CHUNKS)]
            xrs = [xtpool.tile([128, 2, 512], bf16, name=f"xrs{g}")
                   for g in range(CHUNKS)]
            xbs = [xtpool.tile([128, CT, 256], bf16, name=f"xbs{c}")
                   for c in range(CHUNKS)]
            regBs = [cpool.tile([128, 512], bf16, name=f"regBs{g}")
                     for g in range(CHUNKS)]
            XBv = XB[:].rearrange("(t p) n -> p t n", p=128)
            for k in range(2):
                nc.sync.dma_start(m1t[:, k, :], M1T[128 * k:128 * (k + 1), :])
                nc.sync.dma_start(xts[0][:, k, :], XT[128 * k:128 * (k + 1),
                                                      0:512])
                nc.scalar.dma_start(wm[:, k, :], WM[128 * k:128 * (k + 1), :])
            nc.scalar.dma_start(b2v[:], B2V[:])
            nc.scalar.dma_start(b3v[:], B3V[:])
            for g in range(1, CHUNKS):
                qsl = slice(512 * g, 512 * (g + 1))
                eng = nc.sync if g % 2 == 0 else nc.scalar
                for k in range(2):
                    eng.dma_start(xts[g][:, k, :],
                                  XT[128 * k:128 * (k + 1), qsl])
            for c in range(CHUNKS):
                tsl = slice(CT * c, CT * (c + 1))
                nc.gpsimd.dma_start(xbs[c][:], XBv[:, tsl, :])
            nc.sync.dma_start(eye[:], EYE[:])
            nc.sync.dma_start(onesr[:], ONESR[:])
            nc.sync.dma_start(m2bd[:], M2BD[:])
            nc.sync.dma_start(lwbd[:], LWBD[:])
            nc.sync.dma_start(lbm[:], LBM[:])
            nc.sync.dma_start(lbr[:], LBR[:])
            nc.sync.dma_start(gsc[:], GSC[:])

            sp_insts = []
            spEv = spE[:].rearrange("p (t c) -> p c t", c=2)
            spv = sp[:].rearrange("p (t c) -> p c t", c=2)

            # ================= CNN + reg-broadcast phase =================
            with (
                tc.tile_pool(name="cnn", bufs=2) as cnnpool,
                tc.tile_pool(name="pc1", bufs=3, space="PSUM") as pc1pool,
                tc.tile_pool(name="pc2", bufs=1, space="PSUM") as pc2pool,
                tc.tile_pool(name="pc3", bufs=1, space="PSUM") as pc3pool,
            ):
                p3 = pc3pool.tile([128, 2 * TILES], f32)
                for sg in range(NSG):
                    gs_range = range(GPS * sg, GPS * (sg + 1))
                    p2 = pc2pool.tile([128, 512], f32, tag="p2",
                                      name=f"p2sg{sg}")
                    for q, g in enumerate(gs_range):
                        if q % 2 == 0:
                            eh1 = cnnpool.tile([128, 2, 512], f32,
                                               tag="eh1", name=f"eh1p{g}")
                            h1s = cnnpool.tile([128, 2, 512], bf16,
                                               tag="h1s", name=f"h1sp{g}")
                        p1 = pc1pool.tile([128, 512], f32, tag="p1",
                                          name=f"p1g{g}")
                        nc.tensor.matmul(p1[:], m1t[:, 0, :],
                                         xts[g][:, 0, :],
                                         start=True, stop=False)
                        nc.tensor.matmul(p1[:], m1t[:, 1, :],
                                         xts[g][:, 1, :],
                                         start=False, stop=True)
                        nc.scalar.activation(eh1[:, q % 2, :], p1[:],
                                             AF.Exp, bias=b2v[:])
                        if q % 2 == 1:
                            # one merged 1024-wide Ln for the pair of groups
                            nc.scalar.activation(
                                h1s[:].rearrange("p a b -> p (a b)"),
                                eh1[:].rearrange("p a b -> p (a b)"),
                                AF.Ln, bias=1.0)
                            for qq2 in (q - 1, q):
                                nc.tensor.matmul(
                                    p2[32 * qq2:32 * (qq2 + 1), :],
                                    m2bd[:], h1s[:, qq2 % 2, :],
                                    start=True, stop=True,
                                    tile_position=(0, 32 * qq2),
                                    skip_group_check=True)
                    eh2 = cnnpool.tile([128, 512], f32, tag="eh2",
                                       name=f"eh2sg{sg}")
                    nc.scalar.activation(eh2[:], p2[:], AF.Exp,
                                         bias=b3v[:])
                    h2s = cnnpool.tile([128, 512], bf16, tag="h2s",
                                       name=f"h2ssg{sg}")
                    nc.scalar.activation(h2s[:], eh2[:], AF.Ln, bias=1.0)
                    for q, g in enumerate(gs_range):
                        for i in range(4):
                            t = 4 * g + i
                            nc.tensor.matmul(
                                p3[:, 2 * t:2 * t + 2],
                                h2s[32 * q:32 * (q + 1),
                                    128 * i:128 * (i + 1)],
                                lwbd[32 * q:32 * (q + 1), :],
                                start=True, stop=True,
                                tile_position=(32 * q, 0),
                                skip_group_check=True)

                    sgt = slice(4 * GPS * sg, 4 * GPS * (sg + 1))
                    sgs = slice(8 * GPS * sg, 8 * GPS * (sg + 1))
                    nc.scalar.activation(spEv[:, 0, sgt],
                                         p3[:, sgs].rearrange(
                                             "p (t c) -> p c t", c=2)[:, 0, :],
                                         AF.Exp, bias=lbm[:])
                    nc.scalar.activation(spEv[:, 1, sgt],
                                         p3[:, sgs].rearrange(
                                             "p (t c) -> p c t", c=2)[:, 1, :],
                                         AF.Exp, bias=lbr[:])
                    sp_i = nc.scalar.activation(sp[:, sgs], spE[:, sgs],
                                                AF.Ln, bias=1.0)
                    sp_insts.append(sp_i)
                    nc.vector.tensor_scalar(gph[:, sgt], spv[:, 0, sgt],
                                            gsc[:], -0.25,
                                            Alu.mult, Alu.add)
                    nc.vector.tensor_scalar(c13x4[:, sgt], gph[:, sgt],
                                            8.0 / 3.0, 1.0,
                                            Alu.mult, Alu.add)
                    # bf16 copy of reg, then flatten it to one partition via
                    # a small SBUF->SBUF DMA (regf[0, t*128+p] = reg[t,p]).
                    nc.vector.tensor_scalar(regc[:, sgt], spv[:, 1, sgt],
                                            1.0, None, Alu.mult)
                    hsl = slice(BC // NSG * sg, BC // NSG * (sg + 1))
                    pT = pcbpool.tile([16, 128], f32, tag="pt",
                                      name=f"pT{sg}")
                    nc.tensor.matmul(pT[:], regc[:, sgt], eye[:],
                                     start=True, stop=True)
                    regTs = cpool.tile([16, 128], bf16, name=f"regTs{sg}")
                    nc.vector.tensor_scalar(regTs[:], pT[:], 1.0, None,
                                            Alu.mult)
                    nc.sync.dma_start(
                        regf[:, hsl].rearrange("o (t n) -> o t n", n=128),
                        regTs[:].rearrange("t n -> t () n"))


            def emit_bcast_g(g):
                # regB for group g (reg broadcast along partitions) via
                # 1-contract matmuls from regf, then xr = reg (.) x.
                pB = pcbpool.tile([128, 512], f32, tag="pb", name=f"pB{g}")
                for j in range(4):
                    t = 4 * g + j
                    nc.tensor.matmul(
                        pB[:, 128 * j:128 * (j + 1)],
                        onesr[:], regf[0:1, 128 * t:128 * (t + 1)],
                        start=(j == 0), stop=(j == 3),
                        skip_group_check=True)
                nc.scalar.activation(regBs[g][:], pB[:], AF.Copy)
                for k in range(2):
                    nc.vector.tensor_tensor(xrs[g][:, k, :],
                                            xts[g][:, k, :],
                                            regBs[g][:], Alu.mult)

            # ================= main phase =================
            s16_chunks = [None] * CHUNKS
            r_chunks = [None] * CHUNKS
            w_chunks = [None] * CHUNKS
            last_zd = None
            st = {'last_at': None}
            zd_insts = [None] * CHUNKS
            deferred_rw = []
            with tc.tile_pool(name="pm", bufs=3, space="PSUM") as pmpool:
                def emit_main(c):
                    pm = pmpool.tile([128, CF], f32, tag="pm", name=f"pm{c}")
                    # A-matmuls first (only need xt, which lands earliest);
                    # start=True on each 512-col bank's first op resets the
                    # bank, then the identity moves (x_b) and B-matmuls (xr)
                    # accumulate on top.
                    for i in range(CT):
                        t = CT * c + i
                        osl = slice(256 * i, 256 * (i + 1))
                        for k in range(2):
                            nc.tensor.matmul(
                                pm[:, osl],
                                xts[t // 4][:, k,
                                            128 * (t % 4):128 * (t % 4 + 1)],
                                wm[:, k, 0:256],
                                start=(i % 2 == 0 and k == 0), stop=False,
                                skip_group_check=True)
                    for i in range(CT):
                        t = CT * c + i
                        nc.tensor.matmul(pm[:, 256 * i:256 * (i + 1)],
                                         eye[:], xbs[c][:, i, :],
                                         start=False, stop=False,
                                         skip_group_check=True)
                    for i in range(CT):
                        t = CT * c + i
                        osl = slice(256 * i, 256 * (i + 1))
                        for k in range(2):
                            nc.tensor.matmul(
                                pm[:, osl],
                                xrs[t // 4][:, k,
                                            128 * (t % 4):128 * (t % 4 + 1)],
                                wm[:, k, 256:512],
                                start=False, stop=(k == 1),
                                skip_group_check=True)
                    # E2 = (2p - 1/3)^2  [ACT, table-free]
                    e2 = e2pool.tile([128, CF], fp16, tag="e2",
                                     name=f"e2c{c}")
                    nc.scalar.activation(e2[:], pm[:], AF.Square,
                                         scale=2.0, bias=cm13[:])
                    # hm4 = E2 + c13x4 (per-tile scalar; fp16 4x mode)
                    hd = hdpool.tile([128, 2, CF], fp16, tag="hd",
                                     name=f"hd{c}")
                    for i in range(CT):
                        t = CT * c + i
                        osl = slice(256 * i, 256 * (i + 1))
                        nc.vector.tensor_scalar(hd[:, 0, osl], e2[:, osl],
                                                c13x4[:, t:t + 1], None,
                                                Alu.add)
                    # C8, D4
                    c8 = c8pool.tile([128, CF], fp16, tag="c8",
                                     name=f"c8c{c}")
                    nc.vector._custom_dve(
                        COPS['CH8'], out=c8[:], in0=hd[:, 0, :], in1=pm[:],
                        s0=1.0 / 3.0, s1=1.5, imm2=0.5)
                    nc.vector._custom_dve(
                        COPS['DQ'], out=hd[:, 1, :], in0=hd[:, 0, :],
                        in1=c8[:])
                    # s16 = p + 1/3 (ACT Identity; before zd so the last
                    # chunk's s16 doesn't delay the trig table switch)
                    s16 = s16pool.tile([128, CF], fp16, tag="s16",
                                       name=f"s16c{c}")
                    s16_chunks[c] = s16
                    nc.scalar.activation(s16[:], pm[:], AF.Identity,
                                         bias=c13v[:])
                    # z | irs = AbsRsqrt(hm4 | D4)  [one 2048-wide ACT op]
                    zd = rgpool.tile([128, 2, CF], fp16, tag="zd",
                                     name=f"zd{c}")
                    zd_i = nc.scalar.activation(
                        zd[:].rearrange("p a b -> p (a b)"),
                        hd[:].rearrange("p a b -> p (a b)"),
                        AF.Abs_reciprocal_sqrt, bias=epsv[:])
                    if st.get('last_zd') is None:
                        for spi in sp_insts:
                            add_dep_helper(zd_i.ins, spi.ins, sync=True,
                                           reason="absrsqrt after NLE table")
                    elif st.get('last_at') is not None and c == CHUNKS // 2:
                        add_dep_helper(zd_i.ins, st['last_at'].ins,
                                       sync=True,
                                       reason="absrsqrt reload after trig")
                    else:
                        add_dep_helper(zd_i.ins, st['last_zd'].ins,
                                       sync=False, reason="chain zd block")
                    st['last_zd'] = zd_i
                    zd_insts[c] = zd_i
                    # r = hm4*z,  w = C8*irs.  For the last chunks these
                    # are deferred below so the critical hm4->C8->D4->zD
                    # chain of later chunks isn't queued behind them on DVE.
                    if c < CHUNKS // 2:
                        r = rrpool.tile([128, CF], fp16, tag="rr",
                                        name=f"r{c}")
                        r_chunks[c] = r
                        nc.vector.tensor_tensor(r[:], hd[:, 0, :],
                                                zd[:, 0, :], Alu.mult)
                        w = wvpool.tile([128, CF], fp16, tag="wv",
                                        name=f"w{c}")
                        w_chunks[c] = w
                        nc.vector.tensor_tensor(w[:], c8[:], zd[:, 1, :],
                                                Alu.mult)
                    else:
                        deferred_rw.append((c, hd, zd, c8))

                def flush_rw():
                    # all w's first: the next arctan block gates on them,
                    # while the r's aren't needed until RGSIN
                    for (c, hd, zd, c8) in deferred_rw:
                        w = wvpool.tile([128, CF], fp16, tag="wv",
                                        name=f"w{c}")
                        w_chunks[c] = w
                        nc.vector.tensor_tensor(w[:], c8[:], zd[:, 1, :],
                                                Alu.mult)
                    for (c, hd, zd, c8) in deferred_rw:
                        r = rrpool.tile([128, CF], fp16, tag="rr",
                                        name=f"r{c}")
                        r_chunks[c] = r
                        nc.vector.tensor_tensor(r[:], hd[:, 0, :],
                                                zd[:, 0, :], Alu.mult)
                    deferred_rw.clear()

                def emit_tail_at(c, first):
                    w = w_chunks[c]
                    at_i = nc.scalar.activation(w[:], w[:], AF.Arctan)
                    if first:
                        add_dep_helper(at_i.ins, st['last_zd'].ins,
                                       sync=True,
                                       reason="trig block after absrsqrt")
                    else:
                        add_dep_helper(at_i.ins, st['last_at'].ins,
                                       sync=False, reason="chain trig block")
                    st['last_at'] = at_i

                def emit_tail_rest(c):
                    w = w_chunks[c]
                    rg = rgpool.tile([128, CF], fp16, tag="rg",
                                     name=f"rgc{c}")
                    nc.vector._custom_dve(
                        COPS['RGSIN'], out=rg[:], in0=w[:],
                        in1=r_chunks[c][:],
                        s0=-1.0 / 3.0, s1=1.0 / 162.0, imm2=-1.0 / 29160.0)
                    ot = oopool.tile([128, CF], odt, tag="oo", name=f"o{c}")
                    nc.vector.tensor_tensor(ot[:], rg[:], s16_chunks[c][:],
                                            Alu.add)
                    nc.sync.dma_start(
                        OUT[:, CF * c:CF * (c + 1)], ot[:])

                H = CHUNKS // 2
                for c in range(H):
                    emit_bcast_g(c)
                    emit_main(c)
                    if c == H - 1:
                        for g in range(H, CHUNKS):
                            emit_bcast_g(g)
                flush_rw()
                for c in range(H):
                    emit_tail_at(c, first=(c == 0))
                for c in range(H, CHUNKS):
                    emit_main(c)
                flush_rw()
                for c in range(H):
                    emit_tail_rest(c)
                for c in range(H, CHUNKS):
                    emit_tail_at(c, first=(c == H))
                    emit_tail_rest(c)

    nc.compile()
    return nc


def _get_program():
    key = (B_TOTAL, N, N_CORES)
    if key not in _PROG:
        _PROG[key] = _build_program()
    return _PROG[key]


def _host_prep(inputs):
    import ml_dtypes
    bf = ml_dtypes.bfloat16
    x = _np_f32(inputs['x']).reshape(B_TOTAL, N)
    x_b = _np_f32(inputs['x_b']).reshape(B_TOTAL, N)
    m = float(np.asarray(inputs['mass']).reshape(-1)[0])
    gp = float(np.asarray(inputs['gamma_p']).reshape(-1)[0])
    gamma = float(np.log1p(np.exp(gp))) if gp < 30 else gp
    TtT = _np_f32(inputs['TtT'])
    DtD = _np_f32(inputs['DtD'])

    W_A = ((np.eye(N, dtype=np.float32) - np.float32(gamma) * TtT.T)
           / np.float32(3.0 * m)).astype(np.float32)
    W_B = (-np.float32(gamma) * DtD.T / np.float32(3.0 * m)).astype(np.float32)
    WM = np.concatenate([W_A, W_B], axis=1).astype(bf)          # (256,512)

    M1s, M2s, lws = {}, {}, {}
    for tag in ('mu', 'reg'):
        M1s[tag] = _conv_pool_mat(inputs['w2_' + tag], 256)      # (64,256)
        M2s[tag] = _conv_pool_mat(inputs['w3_' + tag], 64)       # (16,64)
        lws[tag] = _np_f32(inputs['lw_' + tag]).reshape(16)
    M1cat = np.concatenate([M1s['mu'], M1s['reg']], axis=0)      # (128,256)
    M1T = np.ascontiguousarray(M1cat.T).astype(bf)               # (256,128)
    M2BD = np.zeros((128, 32), np.float32)
    M2BD[0:64, 0:16] = M2s['mu'].T
    M2BD[64:128, 16:32] = M2s['reg'].T
    M2BD = M2BD.astype(bf)
    LWBD1 = np.zeros((32, 2), np.float32)
    LWBD1[0:16, 0] = lws['mu']
    LWBD1[16:32, 1] = lws['reg']
    LWBD = np.tile(LWBD1, (4, 1)).astype(bf)                     # (128,2)

    def sc(name):
        return float(np.asarray(inputs[name]).reshape(-1)[0])

    B2V = np.full((128, 1), sc('b2_mu'), np.float32)
    B2V[64:] = sc('b2_reg')
    B3V1 = np.full((32, 1), sc('b3_mu'), np.float32)
    B3V1[16:] = sc('b3_reg')
    B3V = np.tile(B3V1, (4, 1))                                  # (128,1)
    LBM = np.full((128, 1), sc('lb_mu'), np.float32)
    LBR = np.full((128, 1), sc('lb_reg'), np.float32)
    GSC = np.full((128, 1), gamma / (m * m), np.float32)

    EYEM = np.eye(128, dtype=np.float32).astype(bf)
    ONES = np.ones((1, 128), np.float32).astype(bf)
    consts = dict(wm=WM, m1t=M1T, m2bd=M2BD, lwbd=LWBD, eye=EYEM,
                  onesr=ONES, b2v=B2V, b3v=B3V, lbm=LBM, lbr=LBR, gsc=GSC)

    xb3 = (np.float32(gamma / (3.0 * m)) * x_b).astype(bf)
    xbf = x.astype(bf)
    in_maps = []
    for c in range(N_CORES):
        rows = slice(BC * c, BC * (c + 1))
        im = dict(consts)
        im['xt'] = np.ascontiguousarray(xbf[rows].T)
        im['xb'] = np.ascontiguousarray(xb3[rows])
        in_maps.append(im)
    return in_maps, m


def kernel(**inputs) -> np.ndarray:
    from concourse import bass_utils
    nc = _get_program()
    in_maps, m = _host_prep(inputs)
    res = bass_utils.run_bass_kernel_spmd(nc, in_maps,
                                          core_ids=list(range(N_CORES)))
    outs = []
    for c in range(N_CORES):
        oc = np.asarray(res.results[c]['out']).reshape(128, TILES, 256)
        outs.append(oc.transpose(1, 0, 2).reshape(BC, 256))
    out = np.concatenate(outs, axis=0).astype(np.float32)
    if m != 1.0:
        out = (np.float32(m) * out).astype(np.float32)
    return np.ascontiguousarray(out.reshape(B_TOTAL, 1, N))
